# revision 50
# baseline (speedup 1.0000x reference)
"""Block-causal (anti-causal: key-block >= query-block) multi-head attention
for Trainium2, run SPMD on 8 NeuronCores.

Problem (hardcoded): B=2, T=8, N=256 (L=2048), D=768, H=12, HD=64.
reference:
    qkv = x @ qkv_w.T + qkv_b ; split into q,k,v heads
    s   = (q @ k.T) / 8 ; mask: query in block ti attends keys in blocks tj >= ti
    p   = softmax(s) ; y = p @ v ; out = y @ proj_w.T + proj_b

Sharding: data-parallel over B (2) x tensor-parallel over heads (4 groups of
3 heads) = 8 cores. Each core computes, for its (batch, head-group):
  - Q^T,K^T = Wqk @ x^T   (bf16 matmuls; bias folded into the PSUM->SBUF
              copy on the DVE: tensor_scalar_add)
  - Vn      = x-chunk.T @ Wv-stream: V in NATURAL layout [keys, 4 slots of
              64] per 128-key chunk (xT chunk stationary, V weights moving).
              Slot 3 is a static all-ones block (memset once): every PV
              matmul's stationary is [v_h | ones] = 128 cols, so PSUM rows
              64:127 of the attention accumulator hold the softmax
              denominator replicated 64x (no partition-broadcast needed).
  - S^T     = K^T.T-chunks vs Q^T   (keys on partitions, queries on free dim)
  - P~      = exp(0.125 * S^T)      (no max-subtraction; logits are tiny)
  - U^T     = [Vn | ones].T @ P~     (rows 0:63 U, rows 64:127 denominator)
  - O^T     = U^T * recip(den) on DVE only (copy den PSUM->SBUF, fast
              reciprocal, broadcast-free multiply)
  - Z^T    += Wproj-slice @ O^T      (partial projection output, bf16)
Host sums the 4 head-group partials per batch and adds proj_b.

DMA: all DRAM tensors are partition-major with per-partition-contiguous
blocks per transfer (cheap DIRECT2D issuance).  Three queues issue in
parallel: sync (x low-dc halves, output), scalar (wqk K/Q + bqk, issued
before its activation-table load), gpsimd (x high-dc halves, wv, bvb,
wqk2, wproj).  K/Q weights and x nt0 land first so the qk chains start
~10us in (vs ~15us with 2-queue serial issuance).

Schedule: K/Q chains for nt0 run first, then the nt0 V-chains and the nt0
qk2 chain (x0-only work that fills the PE while x1..x3 stream); the
remaining qk2/V chains are interleaved with pre-emitted S+exp key-chunk
pairs.  EVERY attention group's S pairs are emitted ~2 groups ahead of
its PV matmuls (uniform lookahead, throttled by the 2-buffer S-tile PSUM
ring).  Key chunks are reordered so the first PV matmul of each group
covers the full 512-query PSUM bank with start=True (pending-zero is
bank-granular).  The masked pair's two 256-query S segments share one
PSUM bank (single accumulation group) so one exp covers both.  The
projection for quarter qq is spread 2-chains-at-a-time between quarter
qq+1's attention groups (the PE then has queued work while each group's
normalization chain resolves on DVE); the otp-half of the last projection
is pre-emitted on the free pmc/pst banks before the final group so only
the K=64 ots-halves + casts + output DMAs (alternating sync/gpsimd
queues) remain after the final normalize.

Empirical notes (HW-verified): a partition-base-shifted custom-DVE
reciprocal output writes garbage (keep recip in/out on one partition
base, split by columns); a K=128 zero-padded ots-half projection is
SLOWER than the K=64 one; pre-emitting a pot-pool proj tile before the
last attention group head-blocks the in-order PE queue.
"""

import functools

import ml_dtypes
import numpy as np

import concourse.bass as bass
import concourse.bacc as bacc_mod
import concourse.mybir as mybir
import concourse.tile as tile
from concourse.bass import ts

F32 = mybir.dt.float32
BF16 = mybir.dt.bfloat16

B, T, N, D = 2, 8, 256, 768
H, HD = 12, 64
L = T * N          # 2048
HPC = 3            # heads per core
NKC = L // 128     # 16 key chunks of 128
NDC = D // 128     # 6 contraction chunks
SCALE = 1.0 / 8.0
VW = 192           # natural-V row width: [v_h0 | v_h1 | v_h2] (ones separate)


DEBUG = False


def group_pairs(qq):
    """Key-chunk pairs for one (head, qq) group; masked pair last."""
    kcs = list(range(4 * qq + 2, 16)) + [4 * qq, 4 * qq + 1]
    return [(kcs[i], kcs[i + 1]) for i in range(0, len(kcs), 2)]


def build_nc():
    nc = bacc_mod.Bacc()

    # x: [128, nt(4), half(2), dc(3), 512] flattened -> per-transfer 3KB
    # contiguous per partition
    xT_d = nc.declare_dram_parameter("xT", [128, 12288], BF16, isOutput=False)
    # wqk: per block (K|Q|qk2) [dc(6), 128] contiguous per partition
    wqkT_d = nc.declare_dram_parameter("wqkT", [128, 2304], BF16, isOutput=False)
    wvT3_d = nc.declare_dram_parameter("wvT3", [128, NDC * VW], BF16, isOutput=False)
    bqk_d = nc.declare_dram_parameter("bqk", [128, 4], F32, isOutput=False)
    bvb_d = nc.declare_dram_parameter("bvb", [128, VW], F32, isOutput=False)
    wprojT_d = nc.declare_dram_parameter("wprojT", [128, 1536], BF16, isOutput=False)
    # z: [128, mc(6), qq(4), 512] flattened
    zT_d = nc.declare_dram_parameter("zT", [128, 12288], BF16, isOutput=True)
    if DEBUG:
        dbg_vn = nc.declare_dram_parameter("dbg_vn", [128, 384], BF16, isOutput=True)
        dbg_qt = nc.declare_dram_parameter("dbg_qt", [128, 512], BF16, isOutput=True)
        dbg_kt = nc.declare_dram_parameter("dbg_kt", [128, 512], BF16, isOutput=True)
        dbg_dt = nc.declare_dram_parameter("dbg_dt", [128, 512], F32, isOutput=True)
        dbg_ot = nc.declare_dram_parameter("dbg_ot", [128, 512], BF16, isOutput=True)

    with tile.TileContext(nc) as tc:
        with (
            tc.tile_pool(name="persist", bufs=1) as pp,
            tc.tile_pool(name="ptile", bufs=26) as ppool,
            tc.tile_pool(name="zbuf", bufs=6) as zpool,
            tc.tile_pool(name="invb", bufs=4) as invpool,
            tc.tile_pool(name="psum_st", bufs=2, space="PSUM") as pst,
            tc.tile_pool(name="psum_ot", bufs=2, space="PSUM") as pot,
            tc.tile_pool(name="psum_mc", bufs=2, space="PSUM") as pmc,
        ):
            # ---- persistent SBUF tensors ----
            wqkT = pp.tile([128, 3, NDC, 128], BF16, tag="wqkT")
            wvT3 = pp.tile([128, NDC, VW], BF16, tag="wvT3")
            bqk = pp.tile([128, 4], F32, tag="bqk")
            bvb = pp.tile([128, VW], F32, tag="bvb")
            wprojT = pp.tile([128, 1536], BF16, tag="wprojT")
            qt = pp.tile([128, L], BF16, tag="qt")      # [q_h0 | q_h1]
            kt = pp.tile([128, L], BF16, tag="kt")      # [k_h0 | k_h1]
            qk2 = pp.tile([128, L], BF16, tag="qk2")    # [q_h2 | k_h2]
            kt2 = pp.tile([64, L], BF16, tag="kt2")     # k_h2 re-based to part 0
            # per key chunk: [v_h0 |ones| v_h1 |ones| v_h2 |ones] so each
            # head's PV stationary [v_h | ones] is one contiguous 128-col
            # slice (the compiler requires single-free-dim weight APs)
            vn = pp.tile([128, NKC, 384], BF16, tag="vn")
            otp = pp.tile([128, L], BF16, tag="otp")    # [o_h0 | o_h1]
            ots = pp.tile([64, L], BF16, tag="ots")     # [o_h2]

            qt_src = [qt[0:64, :], qt[64:128, :], qk2[0:64, :]]
            kt_src = [kt[0:64, :], kt[64:128, :], kt2[0:64, :]]
            ot_dst = [otp[0:64, :], otp[64:128, :], ots[0:64, :]]
            scratch = pp.tile([128, 512], BF16, tag="scratch")

            def pe_warm(n):
                """K=128 dummy matmuls: keep the PE's utilization-driven
                clock ramped across a known stall.  Only safe when no input
                DMA is streaming (the K=128 SBUF reads throttle DMA)."""
                for _ in range(n):
                    ds = pmc.tile([128, 512], F32, tag="qs")
                    nc.tensor.matmul(
                        ds[:], scratch[:, 0:128], scratch[:],
                        start=True, stop=True,
                    )

            def vap(h, kc):
                """PV stationary: [v_h | ones], one contiguous 128-col slice."""
                return vn[:, kc, 128 * h : 128 * h + 128]

            def emit_s_pair(h, qq, pi, a, b):
                """S matmuls + exp for one key-chunk pair; returns a PV job."""
                q_lo = qq * 512
                masked = a == 4 * qq
                st2 = pst.tile([128, 1024], F32, tag="st")
                pt = ppool.tile([128, 1024], BF16, tag="pt")
                if masked:
                    # both 256-query segments adjacent in one PSUM bank ->
                    # a single exp.  One accumulation group (disjoint
                    # regions): pending-zero is bank-granular, so a second
                    # start=True would clobber the first segment.
                    nc.tensor.matmul(
                        st2[:, 0:256],
                        kt_src[h][:, ts(a, 128)],
                        qt_src[h][:, q_lo : q_lo + 256],
                        start=True, stop=False, skip_group_check=True,
                    )
                    nc.tensor.matmul(
                        st2[:, 256:512],
                        kt_src[h][:, ts(b, 128)],
                        qt_src[h][:, q_lo : q_lo + 256],
                        start=False, stop=True, skip_group_check=True,
                    )
                    nc.scalar.activation(
                        pt[:, 0:512],
                        st2[:, 0:512],
                        mybir.ActivationFunctionType.Exp,
                        scale=SCALE,
                    )
                else:
                    nc.tensor.matmul(
                        st2[:, 0:512],
                        kt_src[h][:, ts(a, 128)],
                        qt_src[h][:, q_lo : q_lo + 512],
                        start=True, stop=True,
                    )
                    nc.tensor.matmul(
                        st2[:, 512:1024],
                        kt_src[h][:, ts(b, 128)],
                        qt_src[h][:, q_lo : q_lo + 512],
                        start=True, stop=True,
                    )
                    nc.scalar.activation(
                        pt[:, 0:1024],
                        st2[:, 0:1024],
                        mybir.ActivationFunctionType.Exp,
                        scale=SCALE,
                    )
                return (pi, a, b, pt, masked)

            def emit_pv(h, ot, job):
                pi, a, b, pt, masked = job
                if not masked:
                    nc.tensor.matmul(
                        ot[:, 0:512], vap(h, a), pt[:, 0:512],
                        start=(pi == 0), stop=False, skip_group_check=True,
                    )
                    nc.tensor.matmul(
                        ot[:, 0:512], vap(h, b), pt[:, 512:1024],
                        start=False, stop=False, skip_group_check=True,
                    )
                else:
                    nc.tensor.matmul(
                        ot[:, 0:256], vap(h, a), pt[:, 0:256],
                        start=False, stop=False, skip_group_check=True,
                    )
                    nc.tensor.matmul(
                        ot[:, 0:256], vap(h, b), pt[:, 256:512],
                        start=False, stop=True, skip_group_check=True,
                    )

            # ---- input DMAs ----
            # Three issuing queues in parallel (sync + scalar are HWDGE,
            # gpsimd SWDGE).  Every transfer is per-partition contiguous in
            # DRAM.  Priority: x nt0 + K block + Q block land first.
            with tc.tile_pool(name="xT", bufs=1) as xp:
                xT = xp.tile([128, NDC, L], BF16, tag="xT")

                def x_dma(eng, nt, dc0, dc1):
                    eng.dma_start(
                        out=xT[:, dc0:dc1, ts(nt, 512)],
                        in_=xT_d[:, (nt * 6 + dc0) * 512 : (nt * 6 + dc1) * 512]
                        .rearrange("p (dc w) -> p dc w", w=512),
                    )

                def w_dma(blk, mc):
                    nc.gpsimd.dma_start(
                        out=wqkT[:, mc],
                        in_=wqkT_d[:, 768 * blk : 768 * (blk + 1)]
                        .rearrange("p (dc w) -> p dc w", w=128),
                    )

                # K then Q blocks + bias on the scalar HWDGE queue, issued
                # before the activation-table load (a 3-way x0 split
                # regresses: measured twice)
                x_dma(nc.sync, 0, 0, 3)
                x_dma(nc.gpsimd, 0, 3, 5)
                x_dma(nc.gpsimd, 0, 5, 6)
                nc.scalar.dma_start(
                    out=wqkT[:, 1],
                    in_=wqkT_d[:, 0:768].rearrange("p (dc w) -> p dc w", w=128),
                )
                nc.scalar.dma_start(
                    out=wqkT[:, 0],
                    in_=wqkT_d[:, 768:1536].rearrange("p (dc w) -> p dc w", w=128),
                )
                nc.scalar.dma_start(out=bqk[:], in_=bqk_d[:, :])
                # wv early: the nt0 V-chains fill the PE while x1..x3 stream
                nc.gpsimd.dma_start(
                    out=wvT3[:],
                    in_=wvT3_d[:, :].rearrange("p (dc w) -> p dc w", w=VW),
                )
                nc.gpsimd.dma_start(out=bvb[:], in_=bvb_d[:, :])
                w_dma(2, 2)
                x_dma(nc.sync, 1, 0, 3)
                x_dma(nc.gpsimd, 1, 3, 6)
                x_dma(nc.sync, 2, 0, 3)
                x_dma(nc.gpsimd, 2, 3, 6)
                x_dma(nc.sync, 3, 0, 3)
                x_dma(nc.gpsimd, 3, 3, 6)
                nc.gpsimd.dma_start(out=wprojT[:], in_=wprojT_d[:, :])
                # static tiles (after DMA issuance so they don't delay it)
                nc.vector.memset(scratch[:], 0.0)
                for off in (64, 192, 320):
                    nc.gpsimd.memset(vn[:, :, off : off + 64], 1.0)
                # Pre-warm the exp table: the scalar engine runs exp-only
                # from here on (all bias copies live on DVE).
                warm = zpool.tile([128, 32], F32, tag="warm")
                nc.vector.memset(warm[:], 0.0)
                nc.scalar.activation(
                    warm[:], warm[:], mybir.ActivationFunctionType.Exp
                )

                # ---- phase 1: qk chains + natural-V chains, interleaved with
                # pre-emitted S+exp pairs. ----
                groups_order = [(h, qq) for qq in range(4) for h in range(HPC)]
                pre_jobs = {hq: [] for hq in groups_order}
                s_slots = [
                    (gi, hq, pi, a, b)
                    for gi, hq in enumerate(groups_order)
                    for pi, (a, b) in enumerate(group_pairs(hq[1]))
                ]
                s_done = 0
                kt2_emitted = False

                def emit_next_s(ready_nt, max_gi, limit=1):
                    """Emit queued S pairs whose inputs have landed (kt is
                    written nt-progressively; h2 groups need the kt2 rebase
                    DMA emitted first), up to group index max_gi."""
                    nonlocal s_done
                    while s_done < len(s_slots) and limit > 0:
                        gi, hq, pi, a, b = s_slots[s_done]
                        if gi > max_gi:
                            return
                        if max(a, b) >= 4 * (ready_nt + 1):
                            return
                        if hq[0] == 2 and not kt2_emitted:
                            return
                        pre_jobs[hq].append(emit_s_pair(hq[0], hq[1], pi, a, b))
                        s_done += 1
                        limit -= 1

                def qk_chain(mc, dst, nt):
                    ps = pmc.tile([128, 512], F32, tag="qs")
                    for dc in range(NDC):
                        nc.tensor.matmul(
                            ps[:],
                            wqkT[:, mc, dc, :],
                            xT[:, dc, ts(nt, 512)],
                            start=(dc == 0),
                            stop=(dc == NDC - 1),
                        )
                    nc.vector.tensor_scalar_add(
                        dst[:, ts(nt, 512)], ps[:], bqk[:, mc : mc + 1]
                    )

                def v_chain(kc):
                    vp = pot.tile([128, 256], F32, tag="ot")
                    for dc in range(NDC):
                        nc.tensor.matmul(
                            vp[:, 0:VW],
                            xT[:, dc, ts(kc, 128)],
                            wvT3[:, dc, :],
                            start=(dc == 0),
                            stop=(dc == NDC - 1),
                        )
                    nc.vector.tensor_tensor(
                        out=vn[:, kc, :].rearrange(
                            "p (h s) -> p h s", s=128
                        )[:, :, 0:64],
                        in0=vp[:, 0:VW].rearrange("p (h s) -> p h s", s=64),
                        in1=bvb[:].rearrange("p (h s) -> p h s", s=64),
                        op=mybir.AluOpType.add,
                    )

                def qk2_chain(nt):
                    # qk2 chain (bias on DVE like the rest)
                    ps = pmc.tile([128, 512], F32, tag="qs")
                    for dc in range(NDC):
                        nc.tensor.matmul(
                            ps[:],
                            wqkT[:, 2, dc, :],
                            xT[:, dc, ts(nt, 512)],
                            start=(dc == 0),
                            stop=(dc == NDC - 1),
                        )
                    nc.vector.tensor_scalar_add(
                        qk2[:, ts(nt, 512)], ps[:], bqk[:, 2:3]
                    )

                qk_chain(1, kt, 0)
                qk_chain(0, qt, 0)
                # nt0 V-chains + nt0 qk2 chain need only x0 (+wv/wqk2):
                # they fill the PE while x1..x3 stream in
                for kc in range(4):
                    v_chain(kc)
                qk2_chain(0)
                for nt in range(1, 4):
                    qk_chain(1, kt, nt)       # keys: S pairs consume these
                    emit_next_s(nt - 1, 2, limit=2)
                    qk_chain(0, qt, nt)
                    emit_next_s(nt - 1, 2, limit=2)
                for nt in range(1, 4):
                    qk2_chain(nt)
                    if nt == 3:
                        # k_h2 re-base: partitions 64:128 -> 0:64
                        nc.gpsimd.dma_start(out=kt2[0:64, :], in_=qk2[64:128, :])
                        kt2_emitted = True
                    for kc in range(4 * nt, 4 * nt + 4):
                        v_chain(kc)
                        emit_next_s(3, 2)
                    emit_next_s(3, 2)
                emit_next_s(3, 2, limit=99)  # drain groups 0..2 leftovers
                if DEBUG:
                    nc.sync.dma_start(out=dbg_vn[:, :], in_=vn[:, 0, :])
                    nc.sync.dma_start(out=dbg_qt[:, :], in_=qt[:, 0:512])
                    nc.sync.dma_start(out=dbg_kt[:, :], in_=kt[:, 0:512])

            # ---- attention + interleaved projection ----
            def norm_span(h, ot, dt, q_lo, c0, c1, r0):
                """Normalize ot cols [c0:c1] -> ot_dst cols [q_lo+c0 ...].
                recip in/out must share the partition base on HW (a
                partition-shifted custom-DVE output writes garbage), so the
                inverse goes to a column-offset scratch region [r0...]."""
                nc.vector.tensor_copy(dt[0:64, c0:c1], ot[64:128, c0:c1])
                nc.vector.reciprocal_approx_fast(
                    dt[0:64, r0 : r0 + (c1 - c0)], dt[0:64, c0:c1]
                )
                nc.vector.tensor_tensor(
                    out=ot_dst[h][:, q_lo + c0 : q_lo + c1],
                    in0=ot[0:64, c0:c1],
                    in1=dt[0:64, r0 : r0 + (c1 - c0)],
                    op=mybir.AluOpType.mult,
                )

            def attn_group(h, qq, lookahead_gi):
                ot = pot.tile([128, 512], F32, tag="ot")
                jobs = pre_jobs.pop((h, qq))
                assert len(jobs) == len(group_pairs(qq))
                q_lo = qq * 512
                dt = invpool.tile([64, 1024], F32, tag="dt")
                for job in jobs:
                    emit_pv(h, ot, job)
                    # one lookahead S pair (group +2) per PV slot
                    emit_next_s(3, lookahead_gi, limit=1)
                # normalize: PSUM rows 64:127 hold den replicated; copy to
                # SBUF (custom-DVE recip reading PSUM returns garbage on HW),
                # reciprocal, broadcast-free multiply.  All on DVE.
                norm_span(h, ot, dt, q_lo, 0, 512, 512)
                if DEBUG and h == 0 and qq == 0:
                    nc.sync.dma_start(out=dbg_dt[:, :], in_=dt[:])
                    nc.sync.dma_start(out=dbg_ot[:, :], in_=otp[:, 0:512])

            tail_ps = {}

            def proj_mm1(qq, mc, use_pst=False):
                """First (otp, K=128) half of the mc-th projection chain."""
                if use_pst:
                    # tail: S tiles and attention accumulators are done;
                    # rotate over all three pools (6 banks) so the casts
                    # never gate the next chain
                    if mc % 3 == 0:
                        ps = pmc.tile([128, 512], F32, tag="qs")
                    elif mc % 3 == 1:
                        pst_tile = pst.tile([128, 1024], F32, tag="st", name="pst_tile")
                        ps = pst_tile[:, 0:512]
                    else:
                        ps = pot.tile([128, 512], F32, tag="ot")
                else:
                    # both interleaved proj tiles live on pmc (2 allocations
                    # per group boundary, bufs=2 -> recycle exactly one
                    # boundary back, consumers already drained).  pot then
                    # holds ONLY the attention accumulators in phase 2, so
                    # ot(k+1) recycles ot(k-1) and never waits on the ~2us
                    # normalize of the group right before it.
                    ps = pmc.tile([128, 512], F32, tag="qs")
                nc.tensor.matmul(
                    ps[:],
                    wprojT[:, ts(mc, 128)],
                    otp[:, ts(qq, 512)],
                    start=True, stop=False,
                )
                tail_ps[(qq, mc)] = ps
                return ps

            def proj_mm2(qq, mc, casts_on_scalar=False, cast_eng=None, dma_eng=None):
                """Second (ots, K=64) half + cast + output DMA."""
                ps = tail_ps.pop((qq, mc))
                nc.tensor.matmul(
                    ps[:],
                    wprojT[0:64, 768 + mc * 128 : 768 + (mc + 1) * 128],
                    ots[0:64, ts(qq, 512)],
                    start=False, stop=True,
                )
                zb = zpool.tile([128, 512], BF16, tag="zb")
                # casts_on_scalar: alternate scalar/DVE so the tail casts
                # drain in parallel on two engines
                if cast_eng is not None:
                    if cast_eng is nc.scalar:
                        nc.scalar.copy(zb[:], ps[:])
                    else:
                        cast_eng.tensor_copy(zb[:], ps[:])
                elif casts_on_scalar and mc % 2 == 0:
                    nc.scalar.copy(zb[:], ps[:])
                else:
                    nc.vector.tensor_copy(zb[:], ps[:])
                # alternate output queues so the tail's serial DIRECT2D
                # issuance (~0.6us each) halves
                if dma_eng is None:
                    dma_eng = nc.sync if mc % 2 == 0 else nc.gpsimd
                dma_eng.dma_start(
                    out=zT_d[:, (mc * 4 + qq) * 512 : (mc * 4 + qq + 1) * 512],
                    in_=zb[:],
                )

            def proj(qq, casts_on_scalar=False, use_pst=False):
                for mc in range(NDC):
                    proj_mm1(qq, mc, use_pst=use_pst)
                    proj_mm2(qq, mc, casts_on_scalar=casts_on_scalar)

            # proj(qq-1) is spread 2-chains-at-a-time between the qq groups:
            # the PE then has queued work to run while each group's
            # normalization chain (~2us on DVE) resolves.
            for k, (h, qq) in enumerate(groups_order):
                if h == 2 and qq == 3:
                    # last group, inlined: finish proj(2), run the first PV
                    # pair, THEN pre-emit the otp-half of the last
                    # projection (it waits on h1q3's normalize - emitting it
                    # after this group's first PVs keeps the in-order PE
                    # queue fed while that resolves).  pmc/pst tiles only
                    # (NOT pot: its bufs are needed by this group's
                    # accumulator and the PE queue would deadlock on the
                    # recycle).  Only the K=64 halves remain after the final
                    # normalize.
                    for mc in (4, 5):
                        proj_mm1(2, mc)
                        proj_mm2(2, mc, casts_on_scalar=True)
                    ot = pot.tile([128, 512], F32, tag="ot")
                    jobs = pre_jobs.pop((h, qq))
                    emit_pv(h, ot, jobs[0])
                    emit_pv(h, ot, jobs[1])
                    for mc in (0, 1, 3, 4):
                        proj_mm1(3, mc, use_pst=True)  # pmc/pst tiles
                    dt = invpool.tile([64, 1024], F32, tag="dt")
                    norm_span(h, ot, dt, qq * 512, 0, 512, 512)
                    break
                attn_group(h, qq, min(k + 2, len(groups_order) - 1))
                # drain any stragglers for the next group before its PVs
                emit_next_s(3, min(k + 1, len(groups_order) - 1), limit=99)
                if qq > 0 and not (h == 2 and qq == 3):
                    for mc in (2 * h, 2 * h + 1):
                        proj_mm1(qq - 1, mc)
                        proj_mm2(qq - 1, mc, casts_on_scalar=(qq == 3))
            # mm1(3,2) recycles ot(h1q3) - independent of the final
            # normalize, so it runs while that resolves.  Tail casts spread
            # over scalar/gpsimd/vector and DMA issuance over sync/gpsimd so
            # no single engine serializes the drain; the last chain (mc5)
            # gets the least-loaded engines.
            # (gpsimd cannot read PSUM, so casts go scalar/vector only;
            # scalar takes the final chain, vector is free after the norm)
            tail_eng = {
                0: (nc.scalar, nc.sync),
                1: (nc.vector, nc.gpsimd),
                2: (nc.scalar, nc.sync),
                3: (nc.vector, nc.gpsimd),
                4: (nc.scalar, nc.sync),
                5: (nc.scalar, nc.sync),
            }
            proj_mm1(3, 2, use_pst=True)           # pot tile
            for mc in (0, 1, 2, 3, 4):
                proj_mm2(3, mc, cast_eng=tail_eng[mc][0], dma_eng=tail_eng[mc][1])
            proj_mm1(3, 5, use_pst=True)           # pot tile
            proj_mm2(3, 5, cast_eng=tail_eng[5][0], dma_eng=tail_eng[5][1])

    nc.compile()
    return nc


@functools.lru_cache(maxsize=1)
def get_nc():
    return build_nc()


def make_in_maps(x, qkv_w, qkv_b, proj_w):
    """Per-core host-side sharding/layout prep."""
    x = np.asarray(x, dtype=np.float32)
    qkv_w = np.asarray(qkv_w, dtype=np.float32)
    qkv_b = np.asarray(qkv_b, dtype=np.float32)
    proj_w = np.asarray(proj_w, dtype=np.float32)

    # x host layout: [128, nt, half, dc(3), 512] flattened, per batch
    x_pm = []
    for b in range(B):
        xT = np.ascontiguousarray(x[b].reshape(L, D).T)          # (768, 2048)
        arr = xT.reshape(NDC, 128, 4, 512).transpose(1, 2, 0, 3)  # (128, 4, 6, 512)
        x_pm.append(
            np.ascontiguousarray(arr.reshape(128, 12288)).astype(ml_dtypes.bfloat16)
        )

    in_maps = []
    for c in range(8):
        b, g = divmod(c, 4)
        h0, h1, h2 = 3 * g, 3 * g + 1, 3 * g + 2

        def qrows(h):
            return slice(h * HD, (h + 1) * HD)

        def krows(h):
            return slice(D + h * HD, D + (h + 1) * HD)

        def vrows(h):
            return slice(2 * D + h * HD, 2 * D + (h + 1) * HD)

        # qk selection: mc0=[q0|q1] mc1=[k0|k1] mc2=[q2|k2]
        order = [
            qrows(h0), qrows(h1), krows(h0), krows(h1), qrows(h2), krows(h2),
        ]
        wqk = np.concatenate([qkv_w[s] for s in order], axis=0)       # (384, 768)
        # DRAM layout: K block first, then Q, then qk2 (DMA priority order)
        wqkT_host = np.concatenate(
            [
                pmajor(wqk[128:256].T, 128),   # [k0|k1]
                pmajor(wqk[0:128].T, 128),     # [q0|q1]
                pmajor(wqk[256:384].T, 128),   # [q2|k2]
            ],
            axis=1,
        )
        bqk_sel = np.concatenate([qkv_b[s] for s in order], axis=0)   # (384,)
        bcol = np.zeros((128, 4), np.float32)
        for mc in range(3):
            bcol[:, mc] = bqk_sel[mc * 128 : (mc + 1) * 128]
        # natural-V weights: 3 heads x 64 cols (ones slot is on-chip static)
        wv3 = np.concatenate(
            [qkv_w[vrows(h)] for h in (h0, h1, h2)], axis=0
        )  # (192, 768)
        bv3 = np.concatenate([qkv_b[vrows(h)] for h in (h0, h1, h2)], axis=0)
        wpp = np.concatenate(
            [proj_w[:, ts_np(h0)].T, proj_w[:, ts_np(h1)].T], axis=0
        )  # (128, 768)
        wps = np.concatenate(
            [proj_w[:, ts_np(h2)].T, np.zeros((64, D), np.float32)], axis=0
        )  # (128, 768)
        in_maps.append(
            {
                "xT": x_pm[b],
                "wqkT": wqkT_host,
                "wvT3": pmajor(wv3.T, VW),
                "bqk": bcol,
                "bvb": np.broadcast_to(bv3, (128, VW)).copy(),
                "wprojT": np.ascontiguousarray(
                    np.concatenate([wpp, wps], axis=1)
                ).astype(ml_dtypes.bfloat16),
            }
        )
    return in_maps


def ts_np(h):
    return slice(h * HD, (h + 1) * HD)


def pmajor(wT, width):
    """[768, width] contraction-major -> [128, 6*width] partition-major."""
    w3 = np.asarray(wT, np.float32).reshape(NDC, 128, width)
    return np.ascontiguousarray(
        w3.transpose(1, 0, 2).reshape(128, NDC * width)
    ).astype(ml_dtypes.bfloat16)


def unpack_zT(zt):
    """[128, 12288] device layout -> z^T [768, 2048]."""
    return (
        np.asarray(zt)
        .reshape(128, NDC, 4, 512)
        .transpose(1, 0, 2, 3)
        .reshape(D, L)
    )


def assemble_output(results, proj_b):
    proj_b = np.asarray(proj_b, dtype=np.float32)
    out = np.zeros((B, L, D), np.float32)
    for c in range(8):
        b = c // 4
        out[b] += unpack_zT(results[c]["zT"].astype(np.float32)).T
    out += proj_b[None, None, :]
    return out.reshape(B, T, N, D)


def _install_ntff_hook():
    """The container's antenv stub lacks axon_hooks; recreate it from the
    boot helper so trace=True can profile through libaxon_pjrt."""
    import sys
    import types

    try:
        from antenv.axon_hooks import get_axon_ntff_profile_hook  # noqa: F401

        return
    except ImportError:
        pass
    import antenv
    from trn_agent_boot.trn_boot import _ntff_profile_via_ctypes

    state = {"hook": _ntff_profile_via_ctypes("/opt/axon/libaxon_pjrt.so")}
    mod = types.ModuleType("antenv.axon_hooks")
    mod.set_axon_ntff_profile_hook = lambda h: state.__setitem__("hook", h)
    mod.get_axon_ntff_profile_hook = lambda: state["hook"]
    sys.modules["antenv.axon_hooks"] = mod
    antenv.axon_hooks = mod

    import concourse.bass_utils as bu

    orig_upload = bu.upload_artifacts

    def safe_upload(tmpdir):
        try:
            return orig_upload(tmpdir)
        except Exception:
            return tmpdir

    bu.upload_artifacts = safe_upload


def kernel_with_stats(x, qkv_w, qkv_b, proj_w, proj_b, trace=False):
    from concourse.bass_utils import run_bass_kernel_spmd

    if trace:
        _install_ntff_hook()
    nc = get_nc()
    in_maps = make_in_maps(x, qkv_w, qkv_b, proj_w)
    res = run_bass_kernel_spmd(nc, in_maps, list(range(8)), trace=trace)
    return assemble_output(res.results, proj_b), res


def kernel(x, qkv_w, qkv_b, proj_w, proj_b):
    out, _ = kernel_with_stats(x, qkv_w, qkv_b, proj_w, proj_b)
    return out


# revision 52
# speedup vs baseline: 1.0029x; 1.0029x over previous
"""Block-causal (anti-causal: key-block >= query-block) multi-head attention
for Trainium2, run SPMD on 8 NeuronCores.

Problem (hardcoded): B=2, T=8, N=256 (L=2048), D=768, H=12, HD=64.
reference:
    qkv = x @ qkv_w.T + qkv_b ; split into q,k,v heads
    s   = (q @ k.T) / 8 ; mask: query in block ti attends keys in blocks tj >= ti
    p   = softmax(s) ; y = p @ v ; out = y @ proj_w.T + proj_b

Sharding: data-parallel over B (2) x tensor-parallel over heads (4 groups of
3 heads) = 8 cores. Each core computes, for its (batch, head-group):
  - Q^T,K^T = Wqk @ x^T   (bf16 matmuls; bias folded into the PSUM->SBUF
              copy on the DVE: tensor_scalar_add)
  - Vn      = x-chunk.T @ Wv-stream: V in NATURAL layout [keys, 4 slots of
              64] per 128-key chunk (xT chunk stationary, V weights moving).
              Slot 3 is a static all-ones block (memset once): every PV
              matmul's stationary is [v_h | ones] = 128 cols, so PSUM rows
              64:127 of the attention accumulator hold the softmax
              denominator replicated 64x (no partition-broadcast needed).
  - S^T     = K^T.T-chunks vs Q^T   (keys on partitions, queries on free dim)
  - P~      = exp(0.125 * S^T)      (no max-subtraction; logits are tiny)
  - U^T     = [Vn | ones].T @ P~     (rows 0:63 U, rows 64:127 denominator)
  - O^T     = U^T * recip(den) on DVE only (copy den PSUM->SBUF, fast
              reciprocal, broadcast-free multiply)
  - Z^T    += Wproj-slice @ O^T      (partial projection output, bf16)
Host sums the 4 head-group partials per batch and adds proj_b.

DMA: all DRAM tensors are partition-major with per-partition-contiguous
blocks per transfer (cheap DIRECT2D issuance).  Three queues issue in
parallel: sync (x low-dc halves, output), scalar (wqk K/Q + bqk, issued
before its activation-table load), gpsimd (x high-dc halves, wv, bvb,
wqk2, wproj).  K/Q weights and x nt0 land first so the qk chains start
~10us in (vs ~15us with 2-queue serial issuance).

Schedule: K/Q chains for nt0 run first, then the nt0 V-chains and the nt0
qk2 chain (x0-only work that fills the PE while x1..x3 stream); the
remaining qk2/V chains are interleaved with pre-emitted S+exp key-chunk
pairs.  EVERY attention group's S pairs are emitted ~2 groups ahead of
its PV matmuls (uniform lookahead, throttled by the 2-buffer S-tile PSUM
ring).  Key chunks are reordered so the first PV matmul of each group
covers the full 512-query PSUM bank with start=True (pending-zero is
bank-granular).  The masked pair's two 256-query S segments share one
PSUM bank (single accumulation group) so one exp covers both.  The
projection for quarter qq is spread 2-chains-at-a-time between quarter
qq+1's attention groups (the PE then has queued work while each group's
normalization chain resolves on DVE); the otp-half of the last projection
is pre-emitted on the free pmc/pst banks before the final group so only
the K=64 ots-halves + casts + output DMAs (alternating sync/gpsimd
queues) remain after the final normalize.

Empirical notes (HW-verified): a partition-base-shifted custom-DVE
reciprocal output writes garbage (keep recip in/out on one partition
base, split by columns); a K=128 zero-padded ots-half projection is
SLOWER than the K=64 one; pre-emitting a pot-pool proj tile before the
last attention group head-blocks the in-order PE queue.
"""

import functools

import ml_dtypes
import numpy as np

import concourse.bass as bass
import concourse.bacc as bacc_mod
import concourse.mybir as mybir
import concourse.tile as tile
from concourse.bass import ts

F32 = mybir.dt.float32
BF16 = mybir.dt.bfloat16

B, T, N, D = 2, 8, 256, 768
H, HD = 12, 64
L = T * N          # 2048
HPC = 3            # heads per core
NKC = L // 128     # 16 key chunks of 128
NDC = D // 128     # 6 contraction chunks
SCALE = 1.0 / 8.0
VW = 192           # natural-V row width: [v_h0 | v_h1 | v_h2] (ones separate)


DEBUG = False


def group_pairs(qq):
    """Key-chunk pairs for one (head, qq) group; masked pair last."""
    kcs = list(range(4 * qq + 2, 16)) + [4 * qq, 4 * qq + 1]
    return [(kcs[i], kcs[i + 1]) for i in range(0, len(kcs), 2)]


def build_nc():
    nc = bacc_mod.Bacc()

    # x: [128, nt(4), half(2), dc(3), 512] flattened -> per-transfer 3KB
    # contiguous per partition
    xT_d = nc.declare_dram_parameter("xT", [128, 12288], BF16, isOutput=False)
    # wqk: per block (K|Q|qk2) [dc(6), 128] contiguous per partition
    wqkT_d = nc.declare_dram_parameter("wqkT", [128, 2304], BF16, isOutput=False)
    wvT3_d = nc.declare_dram_parameter("wvT3", [128, NDC * VW], BF16, isOutput=False)
    bqk_d = nc.declare_dram_parameter("bqk", [128, 4], F32, isOutput=False)
    bvb_d = nc.declare_dram_parameter("bvb", [128, VW], F32, isOutput=False)
    wprojT_d = nc.declare_dram_parameter("wprojT", [128, 1536], BF16, isOutput=False)
    # z: [128, mc(6), qq(4), 512] flattened
    zT_d = nc.declare_dram_parameter("zT", [128, 12288], BF16, isOutput=True)
    if DEBUG:
        dbg_vn = nc.declare_dram_parameter("dbg_vn", [128, 384], BF16, isOutput=True)
        dbg_qt = nc.declare_dram_parameter("dbg_qt", [128, 512], BF16, isOutput=True)
        dbg_kt = nc.declare_dram_parameter("dbg_kt", [128, 512], BF16, isOutput=True)
        dbg_dt = nc.declare_dram_parameter("dbg_dt", [128, 512], F32, isOutput=True)
        dbg_ot = nc.declare_dram_parameter("dbg_ot", [128, 512], BF16, isOutput=True)

    with tile.TileContext(nc) as tc:
        with (
            tc.tile_pool(name="persist", bufs=1) as pp,
            tc.tile_pool(name="ptile", bufs=26) as ppool,
            tc.tile_pool(name="zbuf", bufs=6) as zpool,
            tc.tile_pool(name="invb", bufs=2) as invpool,
            tc.tile_pool(name="psum_st", bufs=2, space="PSUM") as pst,
            tc.tile_pool(name="psum_ot", bufs=2, space="PSUM") as pot,
            tc.tile_pool(name="psum_mc", bufs=2, space="PSUM") as pmc,
        ):
            # ---- persistent SBUF tensors ----
            wqkT = pp.tile([128, 3, NDC, 128], BF16, tag="wqkT")
            wvT3 = pp.tile([128, NDC, VW], BF16, tag="wvT3")
            bqk = pp.tile([128, 4], F32, tag="bqk")
            bvb = pp.tile([128, VW], F32, tag="bvb")
            wprojT = pp.tile([128, 1536], BF16, tag="wprojT")
            qt = pp.tile([128, L], BF16, tag="qt")      # [q_h0 | q_h1]
            kt = pp.tile([128, L], BF16, tag="kt")      # [k_h0 | k_h1]
            qk2 = pp.tile([128, L], BF16, tag="qk2")    # [q_h2 | k_h2]
            kt2 = pp.tile([64, L], BF16, tag="kt2")     # k_h2 re-based to part 0
            # per key chunk: [v_h0 |ones| v_h1 |ones| v_h2 |ones] so each
            # head's PV stationary [v_h | ones] is one contiguous 128-col
            # slice (the compiler requires single-free-dim weight APs)
            vn = pp.tile([128, NKC, 384], BF16, tag="vn")
            otp = pp.tile([128, L], BF16, tag="otp")    # [o_h0 | o_h1]
            ots = pp.tile([64, L], BF16, tag="ots")     # [o_h2]

            qt_src = [qt[0:64, :], qt[64:128, :], qk2[0:64, :]]
            kt_src = [kt[0:64, :], kt[64:128, :], kt2[0:64, :]]
            ot_dst = [otp[0:64, :], otp[64:128, :], ots[0:64, :]]
            scratch = pp.tile([128, 512], BF16, tag="scratch")

            def pe_warm(n):
                """K=128 dummy matmuls: keep the PE's utilization-driven
                clock ramped across a known stall.  Only safe when no input
                DMA is streaming (the K=128 SBUF reads throttle DMA)."""
                for _ in range(n):
                    ds = pmc.tile([128, 512], F32, tag="qs")
                    nc.tensor.matmul(
                        ds[:], scratch[:, 0:128], scratch[:],
                        start=True, stop=True,
                    )

            def vap(h, kc):
                """PV stationary: [v_h | ones], one contiguous 128-col slice."""
                return vn[:, kc, 128 * h : 128 * h + 128]

            def emit_s_pair(h, qq, pi, a, b):
                """S matmuls + exp for one key-chunk pair; returns a PV job."""
                q_lo = qq * 512
                masked = a == 4 * qq
                st2 = pst.tile([128, 1024], F32, tag="st")
                pt = ppool.tile([128, 1024], BF16, tag="pt")
                if masked:
                    # both 256-query segments adjacent in one PSUM bank ->
                    # a single exp.  One accumulation group (disjoint
                    # regions): pending-zero is bank-granular, so a second
                    # start=True would clobber the first segment.
                    nc.tensor.matmul(
                        st2[:, 0:256],
                        kt_src[h][:, ts(a, 128)],
                        qt_src[h][:, q_lo : q_lo + 256],
                        start=True, stop=False, skip_group_check=True,
                    )
                    nc.tensor.matmul(
                        st2[:, 256:512],
                        kt_src[h][:, ts(b, 128)],
                        qt_src[h][:, q_lo : q_lo + 256],
                        start=False, stop=True, skip_group_check=True,
                    )
                    nc.scalar.activation(
                        pt[:, 0:512],
                        st2[:, 0:512],
                        mybir.ActivationFunctionType.Exp,
                        scale=SCALE,
                    )
                else:
                    nc.tensor.matmul(
                        st2[:, 0:512],
                        kt_src[h][:, ts(a, 128)],
                        qt_src[h][:, q_lo : q_lo + 512],
                        start=True, stop=True,
                    )
                    nc.tensor.matmul(
                        st2[:, 512:1024],
                        kt_src[h][:, ts(b, 128)],
                        qt_src[h][:, q_lo : q_lo + 512],
                        start=True, stop=True,
                    )
                    nc.scalar.activation(
                        pt[:, 0:1024],
                        st2[:, 0:1024],
                        mybir.ActivationFunctionType.Exp,
                        scale=SCALE,
                    )
                return (pi, a, b, pt, masked)

            def emit_pv(h, ot, job):
                pi, a, b, pt, masked = job
                if not masked:
                    nc.tensor.matmul(
                        ot[:, 0:512], vap(h, a), pt[:, 0:512],
                        start=(pi == 0), stop=False, skip_group_check=True,
                    )
                    nc.tensor.matmul(
                        ot[:, 0:512], vap(h, b), pt[:, 512:1024],
                        start=False, stop=False, skip_group_check=True,
                    )
                else:
                    nc.tensor.matmul(
                        ot[:, 0:256], vap(h, a), pt[:, 0:256],
                        start=False, stop=False, skip_group_check=True,
                    )
                    nc.tensor.matmul(
                        ot[:, 0:256], vap(h, b), pt[:, 256:512],
                        start=False, stop=True, skip_group_check=True,
                    )

            # ---- input DMAs ----
            # Three issuing queues in parallel (sync + scalar are HWDGE,
            # gpsimd SWDGE).  Every transfer is per-partition contiguous in
            # DRAM.  Priority: x nt0 + K block + Q block land first.
            with tc.tile_pool(name="xT", bufs=1) as xp:
                xT = xp.tile([128, NDC, L], BF16, tag="xT")

                def x_dma(eng, nt, dc0, dc1):
                    eng.dma_start(
                        out=xT[:, dc0:dc1, ts(nt, 512)],
                        in_=xT_d[:, (nt * 6 + dc0) * 512 : (nt * 6 + dc1) * 512]
                        .rearrange("p (dc w) -> p dc w", w=512),
                    )

                def w_dma(blk, mc):
                    nc.gpsimd.dma_start(
                        out=wqkT[:, mc],
                        in_=wqkT_d[:, 768 * blk : 768 * (blk + 1)]
                        .rearrange("p (dc w) -> p dc w", w=128),
                    )

                # K then Q blocks + bias on the scalar HWDGE queue, issued
                # before the activation-table load (a 3-way x0 split
                # regresses: measured twice)
                x_dma(nc.sync, 0, 0, 3)
                x_dma(nc.gpsimd, 0, 3, 6)
                nc.scalar.dma_start(
                    out=wqkT[:, 1],
                    in_=wqkT_d[:, 0:768].rearrange("p (dc w) -> p dc w", w=128),
                )
                nc.scalar.dma_start(
                    out=wqkT[:, 0],
                    in_=wqkT_d[:, 768:1536].rearrange("p (dc w) -> p dc w", w=128),
                )
                nc.scalar.dma_start(out=bqk[:], in_=bqk_d[:, :])
                # wv early: the nt0 V-chains fill the PE while x1..x3 stream
                nc.gpsimd.dma_start(
                    out=wvT3[:],
                    in_=wvT3_d[:, :].rearrange("p (dc w) -> p dc w", w=VW),
                )
                nc.gpsimd.dma_start(out=bvb[:], in_=bvb_d[:, :])
                w_dma(2, 2)
                x_dma(nc.sync, 1, 0, 3)
                x_dma(nc.gpsimd, 1, 3, 6)
                x_dma(nc.sync, 2, 0, 3)
                x_dma(nc.gpsimd, 2, 3, 6)
                x_dma(nc.sync, 3, 0, 3)
                x_dma(nc.gpsimd, 3, 3, 6)
                nc.gpsimd.dma_start(out=wprojT[:], in_=wprojT_d[:, :])
                # static tiles (after DMA issuance so they don't delay it)
                nc.vector.memset(scratch[:], 0.0)
                for off in (64, 192, 320):
                    nc.gpsimd.memset(vn[:, :, off : off + 64], 1.0)
                # Pre-warm the exp table: the scalar engine runs exp-only
                # from here on (all bias copies live on DVE).
                warm = zpool.tile([128, 32], F32, tag="warm")
                nc.vector.memset(warm[:], 0.0)
                nc.scalar.activation(
                    warm[:], warm[:], mybir.ActivationFunctionType.Exp
                )

                # ---- phase 1: qk chains + natural-V chains, interleaved with
                # pre-emitted S+exp pairs. ----
                groups_order = [(h, qq) for qq in range(4) for h in range(HPC)]
                pre_jobs = {hq: [] for hq in groups_order}
                s_slots = [
                    (gi, hq, pi, a, b)
                    for gi, hq in enumerate(groups_order)
                    for pi, (a, b) in enumerate(group_pairs(hq[1]))
                ]
                s_done = 0
                kt2_emitted = False

                def emit_next_s(ready_nt, max_gi, limit=1):
                    """Emit queued S pairs whose inputs have landed (kt is
                    written nt-progressively; h2 groups need the kt2 rebase
                    DMA emitted first), up to group index max_gi."""
                    nonlocal s_done
                    while s_done < len(s_slots) and limit > 0:
                        gi, hq, pi, a, b = s_slots[s_done]
                        if gi > max_gi:
                            return
                        if max(a, b) >= 4 * (ready_nt + 1):
                            return
                        if hq[0] == 2 and not kt2_emitted:
                            return
                        pre_jobs[hq].append(emit_s_pair(hq[0], hq[1], pi, a, b))
                        s_done += 1
                        limit -= 1

                def qk_chain(mc, dst, nt):
                    ps = pmc.tile([128, 512], F32, tag="qs")
                    for dc in range(NDC):
                        nc.tensor.matmul(
                            ps[:],
                            wqkT[:, mc, dc, :],
                            xT[:, dc, ts(nt, 512)],
                            start=(dc == 0),
                            stop=(dc == NDC - 1),
                        )
                    nc.vector.tensor_scalar_add(
                        dst[:, ts(nt, 512)], ps[:], bqk[:, mc : mc + 1]
                    )

                def v_chain(kc):
                    vp = pot.tile([128, 256], F32, tag="ot")
                    for dc in range(NDC):
                        nc.tensor.matmul(
                            vp[:, 0:VW],
                            xT[:, dc, ts(kc, 128)],
                            wvT3[:, dc, :],
                            start=(dc == 0),
                            stop=(dc == NDC - 1),
                        )
                    nc.vector.tensor_tensor(
                        out=vn[:, kc, :].rearrange(
                            "p (h s) -> p h s", s=128
                        )[:, :, 0:64],
                        in0=vp[:, 0:VW].rearrange("p (h s) -> p h s", s=64),
                        in1=bvb[:].rearrange("p (h s) -> p h s", s=64),
                        op=mybir.AluOpType.add,
                    )

                def qk2_chain(nt):
                    # qk2 chain (bias on DVE like the rest)
                    ps = pmc.tile([128, 512], F32, tag="qs")
                    for dc in range(NDC):
                        nc.tensor.matmul(
                            ps[:],
                            wqkT[:, 2, dc, :],
                            xT[:, dc, ts(nt, 512)],
                            start=(dc == 0),
                            stop=(dc == NDC - 1),
                        )
                    nc.vector.tensor_scalar_add(
                        qk2[:, ts(nt, 512)], ps[:], bqk[:, 2:3]
                    )

                qk_chain(1, kt, 0)
                qk_chain(0, qt, 0)
                # nt0 V-chains + nt0 qk2 chain need only x0 (+wv/wqk2):
                # they fill the PE while x1..x3 stream in
                for kc in range(4):
                    v_chain(kc)
                qk2_chain(0)
                for nt in range(1, 4):
                    qk_chain(1, kt, nt)       # keys: S pairs consume these
                    emit_next_s(nt - 1, 2, limit=2)
                    qk_chain(0, qt, nt)
                    emit_next_s(nt - 1, 2, limit=2)
                for nt in range(1, 4):
                    qk2_chain(nt)
                    if nt == 3:
                        # k_h2 re-base: partitions 64:128 -> 0:64
                        nc.gpsimd.dma_start(out=kt2[0:64, :], in_=qk2[64:128, :])
                        kt2_emitted = True
                    for kc in range(4 * nt, 4 * nt + 4):
                        v_chain(kc)
                        emit_next_s(3, 2)
                    emit_next_s(3, 2)
                emit_next_s(3, 2, limit=99)  # drain groups 0..2 leftovers
                if DEBUG:
                    nc.sync.dma_start(out=dbg_vn[:, :], in_=vn[:, 0, :])
                    nc.sync.dma_start(out=dbg_qt[:, :], in_=qt[:, 0:512])
                    nc.sync.dma_start(out=dbg_kt[:, :], in_=kt[:, 0:512])

            # ---- attention + interleaved projection ----
            def norm_span(h, ot, dt, q_lo, c0, c1, r0):
                """Normalize ot cols [c0:c1] -> ot_dst cols [q_lo+c0 ...].
                recip in/out must share the partition base on HW (a
                partition-shifted custom-DVE output writes garbage), so the
                inverse goes to a column-offset scratch region [r0...]."""
                nc.vector.tensor_copy(dt[0:64, c0:c1], ot[64:128, c0:c1])
                nc.vector.reciprocal_approx_fast(
                    dt[0:64, r0 : r0 + (c1 - c0)], dt[0:64, c0:c1]
                )
                nc.vector.tensor_tensor(
                    out=ot_dst[h][:, q_lo + c0 : q_lo + c1],
                    in0=ot[0:64, c0:c1],
                    in1=dt[0:64, r0 : r0 + (c1 - c0)],
                    op=mybir.AluOpType.mult,
                )

            def attn_group(h, qq, lookahead_gi):
                ot = pot.tile([128, 512], F32, tag="ot")
                jobs = pre_jobs.pop((h, qq))
                assert len(jobs) == len(group_pairs(qq))
                q_lo = qq * 512
                dt = invpool.tile([64, 1024], F32, tag="dt")
                for job in jobs:
                    emit_pv(h, ot, job)
                    # one lookahead S pair (group +2) per PV slot
                    emit_next_s(3, lookahead_gi, limit=1)
                # normalize: PSUM rows 64:127 hold den replicated; copy to
                # SBUF (custom-DVE recip reading PSUM returns garbage on HW),
                # reciprocal, broadcast-free multiply.  All on DVE.
                norm_span(h, ot, dt, q_lo, 0, 512, 512)
                if DEBUG and h == 0 and qq == 0:
                    nc.sync.dma_start(out=dbg_dt[:, :], in_=dt[:])
                    nc.sync.dma_start(out=dbg_ot[:, :], in_=otp[:, 0:512])

            tail_ps = {}

            def proj_mm1(qq, mc, use_pst=False):
                """First (otp, K=128) half of the mc-th projection chain."""
                if use_pst:
                    # tail: S tiles and attention accumulators are done;
                    # rotate over all three pools (6 banks) so the casts
                    # never gate the next chain
                    if mc % 3 == 0:
                        ps = pmc.tile([128, 512], F32, tag="qs")
                    elif mc % 3 == 1:
                        pst_tile = pst.tile([128, 1024], F32, tag="st", name="pst_tile")
                        ps = pst_tile[:, 0:512]
                    else:
                        ps = pot.tile([128, 512], F32, tag="ot")
                else:
                    # both interleaved proj tiles live on pmc (2 allocations
                    # per group boundary, bufs=2 -> recycle exactly one
                    # boundary back, consumers already drained).  pot then
                    # holds ONLY the attention accumulators in phase 2, so
                    # ot(k+1) recycles ot(k-1) and never waits on the ~2us
                    # normalize of the group right before it.
                    ps = pmc.tile([128, 512], F32, tag="qs")
                nc.tensor.matmul(
                    ps[:],
                    wprojT[:, ts(mc, 128)],
                    otp[:, ts(qq, 512)],
                    start=True, stop=False,
                )
                tail_ps[(qq, mc)] = ps
                return ps

            def proj_mm2(qq, mc, casts_on_scalar=False, cast_eng=None, dma_eng=None):
                """Second (ots, K=64) half + cast + output DMA."""
                ps = tail_ps.pop((qq, mc))
                nc.tensor.matmul(
                    ps[:],
                    wprojT[0:64, 768 + mc * 128 : 768 + (mc + 1) * 128],
                    ots[0:64, ts(qq, 512)],
                    start=False, stop=True,
                )
                zb = zpool.tile([128, 512], BF16, tag="zb")
                # casts_on_scalar: alternate scalar/DVE so the tail casts
                # drain in parallel on two engines
                if cast_eng is not None:
                    if cast_eng is nc.scalar:
                        nc.scalar.copy(zb[:], ps[:])
                    else:
                        cast_eng.tensor_copy(zb[:], ps[:])
                elif casts_on_scalar and mc % 2 == 0:
                    nc.scalar.copy(zb[:], ps[:])
                else:
                    nc.vector.tensor_copy(zb[:], ps[:])
                # alternate output queues so the tail's serial DIRECT2D
                # issuance (~0.6us each) halves
                if dma_eng is None:
                    dma_eng = nc.sync if mc % 2 == 0 else nc.gpsimd
                dma_eng.dma_start(
                    out=zT_d[:, (mc * 4 + qq) * 512 : (mc * 4 + qq + 1) * 512],
                    in_=zb[:],
                )

            def proj(qq, casts_on_scalar=False, use_pst=False):
                for mc in range(NDC):
                    proj_mm1(qq, mc, use_pst=use_pst)
                    proj_mm2(qq, mc, casts_on_scalar=casts_on_scalar)

            # proj(qq-1) is spread 2-chains-at-a-time between the qq groups:
            # the PE then has queued work to run while each group's
            # normalization chain (~2us on DVE) resolves.
            for k, (h, qq) in enumerate(groups_order):
                if h == 2 and qq == 3:
                    # last group, inlined: finish proj(2), run the first PV
                    # pair, THEN pre-emit the otp-half of the last
                    # projection (it waits on h1q3's normalize - emitting it
                    # after this group's first PVs keeps the in-order PE
                    # queue fed while that resolves).  pmc/pst tiles only
                    # (NOT pot: its bufs are needed by this group's
                    # accumulator and the PE queue would deadlock on the
                    # recycle).  Only the K=64 halves remain after the final
                    # normalize.
                    for mc in (4, 5):
                        proj_mm1(2, mc)
                        proj_mm2(2, mc, casts_on_scalar=True)
                    ot = pot.tile([128, 512], F32, tag="ot")
                    jobs = pre_jobs.pop((h, qq))
                    emit_pv(h, ot, jobs[0])
                    emit_pv(h, ot, jobs[1])
                    for mc in (0, 1, 3, 4):
                        proj_mm1(3, mc, use_pst=True)  # pmc/pst tiles
                    dt = invpool.tile([64, 1024], F32, tag="dt")
                    norm_span(h, ot, dt, qq * 512, 0, 512, 512)
                    break
                attn_group(h, qq, min(k + 2, len(groups_order) - 1))
                # drain any stragglers for the next group before its PVs
                emit_next_s(3, min(k + 1, len(groups_order) - 1), limit=99)
                if qq > 0 and not (h == 2 and qq == 3):
                    for mc in (2 * h, 2 * h + 1):
                        proj_mm1(qq - 1, mc)
                        proj_mm2(qq - 1, mc, casts_on_scalar=(qq == 3))
            # mm1(3,2) recycles ot(h1q3) - independent of the final
            # normalize, so it runs while that resolves.  Tail casts spread
            # over scalar/gpsimd/vector and DMA issuance over sync/gpsimd so
            # no single engine serializes the drain; the last chain (mc5)
            # gets the least-loaded engines.
            # (gpsimd cannot read PSUM, so casts go scalar/vector only;
            # scalar takes the final chain, vector is free after the norm)
            tail_eng = {
                0: (nc.scalar, nc.sync),
                1: (nc.vector, nc.gpsimd),
                2: (nc.scalar, nc.sync),
                3: (nc.vector, nc.gpsimd),
                4: (nc.scalar, nc.sync),
                5: (nc.scalar, nc.sync),
            }
            proj_mm1(3, 2, use_pst=True)           # pot tile
            for mc in (0, 1, 2, 3, 4):
                proj_mm2(3, mc, cast_eng=tail_eng[mc][0], dma_eng=tail_eng[mc][1])
            proj_mm1(3, 5, use_pst=True)           # pot tile
            proj_mm2(3, 5, cast_eng=tail_eng[5][0], dma_eng=tail_eng[5][1])

    nc.compile()
    return nc


@functools.lru_cache(maxsize=1)
def get_nc():
    return build_nc()


def make_in_maps(x, qkv_w, qkv_b, proj_w):
    """Per-core host-side sharding/layout prep."""
    x = np.asarray(x, dtype=np.float32)
    qkv_w = np.asarray(qkv_w, dtype=np.float32)
    qkv_b = np.asarray(qkv_b, dtype=np.float32)
    proj_w = np.asarray(proj_w, dtype=np.float32)

    # x host layout: [128, nt, half, dc(3), 512] flattened, per batch
    x_pm = []
    for b in range(B):
        xT = np.ascontiguousarray(x[b].reshape(L, D).T)          # (768, 2048)
        arr = xT.reshape(NDC, 128, 4, 512).transpose(1, 2, 0, 3)  # (128, 4, 6, 512)
        x_pm.append(
            np.ascontiguousarray(arr.reshape(128, 12288)).astype(ml_dtypes.bfloat16)
        )

    in_maps = []
    for c in range(8):
        b, g = divmod(c, 4)
        h0, h1, h2 = 3 * g, 3 * g + 1, 3 * g + 2

        def qrows(h):
            return slice(h * HD, (h + 1) * HD)

        def krows(h):
            return slice(D + h * HD, D + (h + 1) * HD)

        def vrows(h):
            return slice(2 * D + h * HD, 2 * D + (h + 1) * HD)

        # qk selection: mc0=[q0|q1] mc1=[k0|k1] mc2=[q2|k2]
        order = [
            qrows(h0), qrows(h1), krows(h0), krows(h1), qrows(h2), krows(h2),
        ]
        wqk = np.concatenate([qkv_w[s] for s in order], axis=0)       # (384, 768)
        # DRAM layout: K block first, then Q, then qk2 (DMA priority order)
        wqkT_host = np.concatenate(
            [
                pmajor(wqk[128:256].T, 128),   # [k0|k1]
                pmajor(wqk[0:128].T, 128),     # [q0|q1]
                pmajor(wqk[256:384].T, 128),   # [q2|k2]
            ],
            axis=1,
        )
        bqk_sel = np.concatenate([qkv_b[s] for s in order], axis=0)   # (384,)
        bcol = np.zeros((128, 4), np.float32)
        for mc in range(3):
            bcol[:, mc] = bqk_sel[mc * 128 : (mc + 1) * 128]
        # natural-V weights: 3 heads x 64 cols (ones slot is on-chip static)
        wv3 = np.concatenate(
            [qkv_w[vrows(h)] for h in (h0, h1, h2)], axis=0
        )  # (192, 768)
        bv3 = np.concatenate([qkv_b[vrows(h)] for h in (h0, h1, h2)], axis=0)
        wpp = np.concatenate(
            [proj_w[:, ts_np(h0)].T, proj_w[:, ts_np(h1)].T], axis=0
        )  # (128, 768)
        wps = np.concatenate(
            [proj_w[:, ts_np(h2)].T, np.zeros((64, D), np.float32)], axis=0
        )  # (128, 768)
        in_maps.append(
            {
                "xT": x_pm[b],
                "wqkT": wqkT_host,
                "wvT3": pmajor(wv3.T, VW),
                "bqk": bcol,
                "bvb": np.broadcast_to(bv3, (128, VW)).copy(),
                "wprojT": np.ascontiguousarray(
                    np.concatenate([wpp, wps], axis=1)
                ).astype(ml_dtypes.bfloat16),
            }
        )
    return in_maps


def ts_np(h):
    return slice(h * HD, (h + 1) * HD)


def pmajor(wT, width):
    """[768, width] contraction-major -> [128, 6*width] partition-major."""
    w3 = np.asarray(wT, np.float32).reshape(NDC, 128, width)
    return np.ascontiguousarray(
        w3.transpose(1, 0, 2).reshape(128, NDC * width)
    ).astype(ml_dtypes.bfloat16)


def unpack_zT(zt):
    """[128, 12288] device layout -> z^T [768, 2048]."""
    return (
        np.asarray(zt)
        .reshape(128, NDC, 4, 512)
        .transpose(1, 0, 2, 3)
        .reshape(D, L)
    )


def assemble_output(results, proj_b):
    proj_b = np.asarray(proj_b, dtype=np.float32)
    out = np.zeros((B, L, D), np.float32)
    for c in range(8):
        b = c // 4
        out[b] += unpack_zT(results[c]["zT"].astype(np.float32)).T
    out += proj_b[None, None, :]
    return out.reshape(B, T, N, D)


def _install_ntff_hook():
    """The container's antenv stub lacks axon_hooks; recreate it from the
    boot helper so trace=True can profile through libaxon_pjrt."""
    import sys
    import types

    try:
        from antenv.axon_hooks import get_axon_ntff_profile_hook  # noqa: F401

        return
    except ImportError:
        pass
    import antenv
    from trn_agent_boot.trn_boot import _ntff_profile_via_ctypes

    state = {"hook": _ntff_profile_via_ctypes("/opt/axon/libaxon_pjrt.so")}
    mod = types.ModuleType("antenv.axon_hooks")
    mod.set_axon_ntff_profile_hook = lambda h: state.__setitem__("hook", h)
    mod.get_axon_ntff_profile_hook = lambda: state["hook"]
    sys.modules["antenv.axon_hooks"] = mod
    antenv.axon_hooks = mod

    import concourse.bass_utils as bu

    orig_upload = bu.upload_artifacts

    def safe_upload(tmpdir):
        try:
            return orig_upload(tmpdir)
        except Exception:
            return tmpdir

    bu.upload_artifacts = safe_upload


def kernel_with_stats(x, qkv_w, qkv_b, proj_w, proj_b, trace=False):
    from concourse.bass_utils import run_bass_kernel_spmd

    if trace:
        _install_ntff_hook()
    nc = get_nc()
    in_maps = make_in_maps(x, qkv_w, qkv_b, proj_w)
    res = run_bass_kernel_spmd(nc, in_maps, list(range(8)), trace=trace)
    return assemble_output(res.results, proj_b), res


def kernel(x, qkv_w, qkv_b, proj_w, proj_b):
    out, _ = kernel_with_stats(x, qkv_w, qkv_b, proj_w, proj_b)
    return out


# revision 53
# speedup vs baseline: 1.0096x; 1.0067x over previous
"""Block-causal (anti-causal: key-block >= query-block) multi-head attention
for Trainium2, run SPMD on 8 NeuronCores.

Problem (hardcoded): B=2, T=8, N=256 (L=2048), D=768, H=12, HD=64.
reference:
    qkv = x @ qkv_w.T + qkv_b ; split into q,k,v heads
    s   = (q @ k.T) / 8 ; mask: query in block ti attends keys in blocks tj >= ti
    p   = softmax(s) ; y = p @ v ; out = y @ proj_w.T + proj_b

Sharding: data-parallel over B (2) x tensor-parallel over heads (4 groups of
3 heads) = 8 cores. Each core computes, for its (batch, head-group):
  - Q^T,K^T = Wqk @ x^T   (bf16 matmuls; bias folded into the PSUM->SBUF
              copy on the DVE: tensor_scalar_add)
  - Vn      = x-chunk.T @ Wv-stream: V in NATURAL layout [keys, 4 slots of
              64] per 128-key chunk (xT chunk stationary, V weights moving).
              Slot 3 is a static all-ones block (memset once): every PV
              matmul's stationary is [v_h | ones] = 128 cols, so PSUM rows
              64:127 of the attention accumulator hold the softmax
              denominator replicated 64x (no partition-broadcast needed).
  - S^T     = K^T.T-chunks vs Q^T   (keys on partitions, queries on free dim)
  - P~      = exp(0.125 * S^T)      (no max-subtraction; logits are tiny)
  - U^T     = [Vn | ones].T @ P~     (rows 0:63 U, rows 64:127 denominator)
  - O^T     = U^T * recip(den) on DVE only (copy den PSUM->SBUF, fast
              reciprocal, broadcast-free multiply)
  - Z^T    += Wproj-slice @ O^T      (partial projection output, bf16)
Host sums the 4 head-group partials per batch and adds proj_b.

DMA: all DRAM tensors are partition-major with per-partition-contiguous
blocks per transfer (cheap DIRECT2D issuance).  Three queues issue in
parallel: sync (x low-dc halves, output), scalar (wqk K/Q + bqk, issued
before its activation-table load), gpsimd (x high-dc halves, wv, bvb,
wqk2, wproj).  K/Q weights and x nt0 land first so the qk chains start
~10us in (vs ~15us with 2-queue serial issuance).

Schedule: K/Q chains for nt0 run first, then the nt0 V-chains and the nt0
qk2 chain (x0-only work that fills the PE while x1..x3 stream); the
remaining qk2/V chains are interleaved with pre-emitted S+exp key-chunk
pairs.  EVERY attention group's S pairs are emitted ~2 groups ahead of
its PV matmuls (uniform lookahead, throttled by the 2-buffer S-tile PSUM
ring).  Key chunks are reordered so the first PV matmul of each group
covers the full 512-query PSUM bank with start=True (pending-zero is
bank-granular).  The masked pair's two 256-query S segments share one
PSUM bank (single accumulation group) so one exp covers both.  The
projection for quarter qq is spread 2-chains-at-a-time between quarter
qq+1's attention groups (the PE then has queued work while each group's
normalization chain resolves on DVE); the otp-half of the last projection
is pre-emitted on the free pmc/pst banks before the final group so only
the K=64 ots-halves + casts + output DMAs (alternating sync/gpsimd
queues) remain after the final normalize.

Empirical notes (HW-verified): a partition-base-shifted custom-DVE
reciprocal output writes garbage (keep recip in/out on one partition
base, split by columns); a K=128 zero-padded ots-half projection is
SLOWER than the K=64 one; pre-emitting a pot-pool proj tile before the
last attention group head-blocks the in-order PE queue.
"""

import functools

import ml_dtypes
import numpy as np

import concourse.bass as bass
import concourse.bacc as bacc_mod
import concourse.mybir as mybir
import concourse.tile as tile
from concourse.bass import ts

F32 = mybir.dt.float32
BF16 = mybir.dt.bfloat16

B, T, N, D = 2, 8, 256, 768
H, HD = 12, 64
L = T * N          # 2048
HPC = 3            # heads per core
NKC = L // 128     # 16 key chunks of 128
NDC = D // 128     # 6 contraction chunks
SCALE = 1.0 / 8.0
VW = 192           # natural-V row width: [v_h0 | v_h1 | v_h2] (ones separate)


DEBUG = False


def group_pairs(qq):
    """Key-chunk pairs for one (head, qq) group; masked pair last."""
    kcs = list(range(4 * qq + 2, 16)) + [4 * qq, 4 * qq + 1]
    return [(kcs[i], kcs[i + 1]) for i in range(0, len(kcs), 2)]


def build_nc():
    nc = bacc_mod.Bacc()

    # x: [128, nt(4), half(2), dc(3), 512] flattened -> per-transfer 3KB
    # contiguous per partition
    xT_d = nc.declare_dram_parameter("xT", [128, 12288], BF16, isOutput=False)
    # wqk: per block (K|Q|qk2) [dc(6), 128] contiguous per partition
    wqkT_d = nc.declare_dram_parameter("wqkT", [128, 2304], BF16, isOutput=False)
    wvT3_d = nc.declare_dram_parameter("wvT3", [128, NDC * VW], BF16, isOutput=False)
    bqk_d = nc.declare_dram_parameter("bqk", [128, 4], F32, isOutput=False)
    bvb_d = nc.declare_dram_parameter("bvb", [128, VW], F32, isOutput=False)
    wprojT_d = nc.declare_dram_parameter("wprojT", [128, 1536], BF16, isOutput=False)
    # z: [128, mc(6), qq(4), 512] flattened
    zT_d = nc.declare_dram_parameter("zT", [128, 12288], BF16, isOutput=True)
    if DEBUG:
        dbg_vn = nc.declare_dram_parameter("dbg_vn", [128, 384], BF16, isOutput=True)
        dbg_qt = nc.declare_dram_parameter("dbg_qt", [128, 512], BF16, isOutput=True)
        dbg_kt = nc.declare_dram_parameter("dbg_kt", [128, 512], BF16, isOutput=True)
        dbg_dt = nc.declare_dram_parameter("dbg_dt", [128, 512], F32, isOutput=True)
        dbg_ot = nc.declare_dram_parameter("dbg_ot", [128, 512], BF16, isOutput=True)

    with tile.TileContext(nc) as tc:
        with (
            tc.tile_pool(name="persist", bufs=1) as pp,
            tc.tile_pool(name="ptile", bufs=26) as ppool,
            tc.tile_pool(name="zbuf", bufs=6) as zpool,
            tc.tile_pool(name="invb", bufs=2) as invpool,
            tc.tile_pool(name="psum_st", bufs=2, space="PSUM") as pst,
            tc.tile_pool(name="psum_ot", bufs=2, space="PSUM") as pot,
            tc.tile_pool(name="psum_mc", bufs=2, space="PSUM") as pmc,
        ):
            # ---- persistent SBUF tensors ----
            wqkT = pp.tile([128, 3, NDC, 128], BF16, tag="wqkT")
            wvT3 = pp.tile([128, NDC, VW], BF16, tag="wvT3")
            bqk = pp.tile([128, 4], F32, tag="bqk")
            bvb = pp.tile([128, VW], F32, tag="bvb")
            wprojT = pp.tile([128, 1536], BF16, tag="wprojT")
            qt = pp.tile([128, L], BF16, tag="qt")      # [q_h0 | q_h1]
            kt = pp.tile([128, L], BF16, tag="kt")      # [k_h0 | k_h1]
            qk2 = pp.tile([128, L], BF16, tag="qk2")    # [q_h2 | k_h2]
            kt2 = pp.tile([64, L], BF16, tag="kt2")     # k_h2 re-based to part 0
            # per key chunk: [v_h0 |ones| v_h1 |ones| v_h2 |ones] so each
            # head's PV stationary [v_h | ones] is one contiguous 128-col
            # slice (the compiler requires single-free-dim weight APs)
            vn = pp.tile([128, NKC, 384], BF16, tag="vn")
            otp = pp.tile([128, L], BF16, tag="otp")    # [o_h0 | o_h1]
            ots = pp.tile([64, L], BF16, tag="ots")     # [o_h2]

            qt_src = [qt[0:64, :], qt[64:128, :], qk2[0:64, :]]
            kt_src = [kt[0:64, :], kt[64:128, :], kt2[0:64, :]]
            ot_dst = [otp[0:64, :], otp[64:128, :], ots[0:64, :]]
            scratch = pp.tile([128, 512], BF16, tag="scratch")

            def pe_warm(n):
                """K=128 dummy matmuls: keep the PE's utilization-driven
                clock ramped across a known stall.  Only safe when no input
                DMA is streaming (the K=128 SBUF reads throttle DMA)."""
                for _ in range(n):
                    ds = pmc.tile([128, 512], F32, tag="qs")
                    nc.tensor.matmul(
                        ds[:], scratch[:, 0:128], scratch[:],
                        start=True, stop=True,
                    )

            def vap(h, kc):
                """PV stationary: [v_h | ones], one contiguous 128-col slice."""
                return vn[:, kc, 128 * h : 128 * h + 128]

            def emit_s_pair(h, qq, pi, a, b):
                """S matmuls + exp for one key-chunk pair; returns a PV job."""
                q_lo = qq * 512
                masked = a == 4 * qq
                st2 = pst.tile([128, 1024], F32, tag="st")
                pt = ppool.tile([128, 1024], BF16, tag="pt")
                if masked:
                    # both 256-query segments adjacent in one PSUM bank ->
                    # a single exp.  One accumulation group (disjoint
                    # regions): pending-zero is bank-granular, so a second
                    # start=True would clobber the first segment.
                    nc.tensor.matmul(
                        st2[:, 0:256],
                        kt_src[h][:, ts(a, 128)],
                        qt_src[h][:, q_lo : q_lo + 256],
                        start=True, stop=False, skip_group_check=True,
                    )
                    nc.tensor.matmul(
                        st2[:, 256:512],
                        kt_src[h][:, ts(b, 128)],
                        qt_src[h][:, q_lo : q_lo + 256],
                        start=False, stop=True, skip_group_check=True,
                    )
                    nc.scalar.activation(
                        pt[:, 0:512],
                        st2[:, 0:512],
                        mybir.ActivationFunctionType.Exp,
                        scale=SCALE,
                    )
                else:
                    nc.tensor.matmul(
                        st2[:, 0:512],
                        kt_src[h][:, ts(a, 128)],
                        qt_src[h][:, q_lo : q_lo + 512],
                        start=True, stop=True,
                    )
                    nc.tensor.matmul(
                        st2[:, 512:1024],
                        kt_src[h][:, ts(b, 128)],
                        qt_src[h][:, q_lo : q_lo + 512],
                        start=True, stop=True,
                    )
                    nc.scalar.activation(
                        pt[:, 0:1024],
                        st2[:, 0:1024],
                        mybir.ActivationFunctionType.Exp,
                        scale=SCALE,
                    )
                return (pi, a, b, pt, masked)

            def emit_pv(h, ot, job):
                pi, a, b, pt, masked = job
                if not masked:
                    nc.tensor.matmul(
                        ot[:, 0:512], vap(h, a), pt[:, 0:512],
                        start=(pi == 0), stop=False, skip_group_check=True,
                    )
                    nc.tensor.matmul(
                        ot[:, 0:512], vap(h, b), pt[:, 512:1024],
                        start=False, stop=False, skip_group_check=True,
                    )
                else:
                    nc.tensor.matmul(
                        ot[:, 0:256], vap(h, a), pt[:, 0:256],
                        start=False, stop=False, skip_group_check=True,
                    )
                    nc.tensor.matmul(
                        ot[:, 0:256], vap(h, b), pt[:, 256:512],
                        start=False, stop=True, skip_group_check=True,
                    )

            # ---- input DMAs ----
            # Three issuing queues in parallel (sync + scalar are HWDGE,
            # gpsimd SWDGE).  Every transfer is per-partition contiguous in
            # DRAM.  Priority: x nt0 + K block + Q block land first.
            with tc.tile_pool(name="xT", bufs=1) as xp:
                xT = xp.tile([128, NDC, L], BF16, tag="xT")

                def x_dma(eng, nt, dc0, dc1):
                    eng.dma_start(
                        out=xT[:, dc0:dc1, ts(nt, 512)],
                        in_=xT_d[:, (nt * 6 + dc0) * 512 : (nt * 6 + dc1) * 512]
                        .rearrange("p (dc w) -> p dc w", w=512),
                    )

                def w_dma(blk, mc):
                    nc.gpsimd.dma_start(
                        out=wqkT[:, mc],
                        in_=wqkT_d[:, 768 * blk : 768 * (blk + 1)]
                        .rearrange("p (dc w) -> p dc w", w=128),
                    )

                # K then Q blocks + bias on the scalar HWDGE queue, issued
                # before the activation-table load (a 3-way x0 split
                # regresses: measured twice)
                x_dma(nc.sync, 0, 0, 3)
                x_dma(nc.gpsimd, 0, 3, 6)
                nc.scalar.dma_start(
                    out=wqkT[:, 1],
                    in_=wqkT_d[:, 0:768].rearrange("p (dc w) -> p dc w", w=128),
                )
                nc.scalar.dma_start(
                    out=wqkT[:, 0],
                    in_=wqkT_d[:, 768:1536].rearrange("p (dc w) -> p dc w", w=128),
                )
                nc.scalar.dma_start(out=bqk[:], in_=bqk_d[:, :])
                # wv early: the nt0 V-chains fill the PE while x1..x3 stream
                nc.gpsimd.dma_start(
                    out=wvT3[:],
                    in_=wvT3_d[:, :].rearrange("p (dc w) -> p dc w", w=VW),
                )
                nc.gpsimd.dma_start(out=bvb[:], in_=bvb_d[:, :])
                w_dma(2, 2)
                x_dma(nc.sync, 1, 0, 3)
                x_dma(nc.gpsimd, 1, 3, 6)
                x_dma(nc.sync, 2, 0, 3)
                x_dma(nc.gpsimd, 2, 3, 6)
                x_dma(nc.sync, 3, 0, 3)
                x_dma(nc.gpsimd, 3, 3, 6)
                nc.gpsimd.dma_start(out=wprojT[:], in_=wprojT_d[:, :])
                # static tiles (after DMA issuance so they don't delay it)
                nc.vector.memset(scratch[:], 0.0)
                for off in (64, 192, 320):
                    nc.gpsimd.memset(vn[:, :, off : off + 64], 1.0)
                # Pre-warm the exp table: the scalar engine runs exp-only
                # from here on (all bias copies live on DVE).
                warm = zpool.tile([128, 32], F32, tag="warm")
                nc.vector.memset(warm[:], 0.0)
                nc.scalar.activation(
                    warm[:], warm[:], mybir.ActivationFunctionType.Exp
                )

                # ---- phase 1: qk chains + natural-V chains, interleaved with
                # pre-emitted S+exp pairs. ----
                groups_order = [(h, qq) for qq in range(4) for h in range(HPC)]
                pre_jobs = {hq: [] for hq in groups_order}
                s_slots = [
                    (gi, hq, pi, a, b)
                    for gi, hq in enumerate(groups_order)
                    for pi, (a, b) in enumerate(group_pairs(hq[1]))
                ]
                s_done = 0
                kt2_emitted = False

                def emit_next_s(ready_nt, max_gi, limit=1):
                    """Emit queued S pairs whose inputs have landed (kt is
                    written nt-progressively; h2 groups need the kt2 rebase
                    DMA emitted first), up to group index max_gi."""
                    nonlocal s_done
                    while s_done < len(s_slots) and limit > 0:
                        gi, hq, pi, a, b = s_slots[s_done]
                        if gi > max_gi:
                            return
                        if max(a, b) >= 4 * (ready_nt + 1):
                            return
                        if hq[0] == 2 and not kt2_emitted:
                            return
                        pre_jobs[hq].append(emit_s_pair(hq[0], hq[1], pi, a, b))
                        s_done += 1
                        limit -= 1

                def qk_chain(mc, dst, nt):
                    ps = pmc.tile([128, 512], F32, tag="qs")
                    for dc in range(NDC):
                        nc.tensor.matmul(
                            ps[:],
                            wqkT[:, mc, dc, :],
                            xT[:, dc, ts(nt, 512)],
                            start=(dc == 0),
                            stop=(dc == NDC - 1),
                        )
                    nc.vector.tensor_scalar_add(
                        dst[:, ts(nt, 512)], ps[:], bqk[:, mc : mc + 1]
                    )

                def v_chain(kc):
                    vp = pot.tile([128, 256], F32, tag="ot")
                    for dc in range(NDC):
                        nc.tensor.matmul(
                            vp[:, 0:VW],
                            xT[:, dc, ts(kc, 128)],
                            wvT3[:, dc, :],
                            start=(dc == 0),
                            stop=(dc == NDC - 1),
                        )
                    nc.vector.tensor_tensor(
                        out=vn[:, kc, :].rearrange(
                            "p (h s) -> p h s", s=128
                        )[:, :, 0:64],
                        in0=vp[:, 0:VW].rearrange("p (h s) -> p h s", s=64),
                        in1=bvb[:].rearrange("p (h s) -> p h s", s=64),
                        op=mybir.AluOpType.add,
                    )

                def qk2_chain(nt):
                    # qk2 chain (bias on DVE like the rest)
                    ps = pmc.tile([128, 512], F32, tag="qs")
                    for dc in range(NDC):
                        nc.tensor.matmul(
                            ps[:],
                            wqkT[:, 2, dc, :],
                            xT[:, dc, ts(nt, 512)],
                            start=(dc == 0),
                            stop=(dc == NDC - 1),
                        )
                    nc.vector.tensor_scalar_add(
                        qk2[:, ts(nt, 512)], ps[:], bqk[:, 2:3]
                    )

                # small warm-up matmuls (only need the scratch memset): start
                # the PE's HAM activity window ~1us before x0 lands so the
                # first chains reach the 2.4GHz clock sooner.  N=256 keeps
                # the SBUF read traffic low (large dummies throttle the
                # input DMA stream - measured in an earlier session).
                for _ in range(6):
                    ds = pmc.tile([128, 512], F32, tag="qs")
                    nc.tensor.matmul(
                        ds[:, 0:256], scratch[:, 0:128], scratch[:, 0:256],
                        start=True, stop=True,
                    )
                qk_chain(1, kt, 0)
                qk_chain(0, qt, 0)
                # nt0 V-chains + nt0 qk2 chain need only x0 (+wv/wqk2):
                # they fill the PE while x1..x3 stream in
                for kc in range(4):
                    v_chain(kc)
                qk2_chain(0)
                for nt in range(1, 4):
                    qk_chain(1, kt, nt)       # keys: S pairs consume these
                    emit_next_s(nt - 1, 2, limit=2)
                    qk_chain(0, qt, nt)
                    emit_next_s(nt - 1, 2, limit=2)
                for nt in range(1, 4):
                    qk2_chain(nt)
                    if nt == 3:
                        # k_h2 re-base: partitions 64:128 -> 0:64
                        nc.gpsimd.dma_start(out=kt2[0:64, :], in_=qk2[64:128, :])
                        kt2_emitted = True
                    for kc in range(4 * nt, 4 * nt + 4):
                        v_chain(kc)
                        emit_next_s(3, 2)
                    emit_next_s(3, 2)
                emit_next_s(3, 2, limit=99)  # drain groups 0..2 leftovers
                if DEBUG:
                    nc.sync.dma_start(out=dbg_vn[:, :], in_=vn[:, 0, :])
                    nc.sync.dma_start(out=dbg_qt[:, :], in_=qt[:, 0:512])
                    nc.sync.dma_start(out=dbg_kt[:, :], in_=kt[:, 0:512])

            # ---- attention + interleaved projection ----
            def norm_span(h, ot, dt, q_lo, c0, c1, r0):
                """Normalize ot cols [c0:c1] -> ot_dst cols [q_lo+c0 ...].
                recip in/out must share the partition base on HW (a
                partition-shifted custom-DVE output writes garbage), so the
                inverse goes to a column-offset scratch region [r0...]."""
                nc.vector.tensor_copy(dt[0:64, c0:c1], ot[64:128, c0:c1])
                nc.vector.reciprocal_approx_fast(
                    dt[0:64, r0 : r0 + (c1 - c0)], dt[0:64, c0:c1]
                )
                nc.vector.tensor_tensor(
                    out=ot_dst[h][:, q_lo + c0 : q_lo + c1],
                    in0=ot[0:64, c0:c1],
                    in1=dt[0:64, r0 : r0 + (c1 - c0)],
                    op=mybir.AluOpType.mult,
                )

            def attn_group(h, qq, lookahead_gi):
                ot = pot.tile([128, 512], F32, tag="ot")
                jobs = pre_jobs.pop((h, qq))
                assert len(jobs) == len(group_pairs(qq))
                q_lo = qq * 512
                dt = invpool.tile([64, 1024], F32, tag="dt")
                for job in jobs:
                    emit_pv(h, ot, job)
                    # one lookahead S pair (group +2) per PV slot
                    emit_next_s(3, lookahead_gi, limit=1)
                # normalize: PSUM rows 64:127 hold den replicated; copy to
                # SBUF (custom-DVE recip reading PSUM returns garbage on HW),
                # reciprocal, broadcast-free multiply.  All on DVE.
                norm_span(h, ot, dt, q_lo, 0, 512, 512)
                if DEBUG and h == 0 and qq == 0:
                    nc.sync.dma_start(out=dbg_dt[:, :], in_=dt[:])
                    nc.sync.dma_start(out=dbg_ot[:, :], in_=otp[:, 0:512])

            tail_ps = {}

            def proj_mm1(qq, mc, use_pst=False):
                """First (otp, K=128) half of the mc-th projection chain."""
                if use_pst:
                    # tail: S tiles and attention accumulators are done;
                    # rotate over all three pools (6 banks) so the casts
                    # never gate the next chain
                    if mc % 3 == 0:
                        ps = pmc.tile([128, 512], F32, tag="qs")
                    elif mc % 3 == 1:
                        pst_tile = pst.tile([128, 1024], F32, tag="st", name="pst_tile")
                        ps = pst_tile[:, 0:512]
                    else:
                        ps = pot.tile([128, 512], F32, tag="ot")
                else:
                    # both interleaved proj tiles live on pmc (2 allocations
                    # per group boundary, bufs=2 -> recycle exactly one
                    # boundary back, consumers already drained).  pot then
                    # holds ONLY the attention accumulators in phase 2, so
                    # ot(k+1) recycles ot(k-1) and never waits on the ~2us
                    # normalize of the group right before it.
                    ps = pmc.tile([128, 512], F32, tag="qs")
                nc.tensor.matmul(
                    ps[:],
                    wprojT[:, ts(mc, 128)],
                    otp[:, ts(qq, 512)],
                    start=True, stop=False,
                )
                tail_ps[(qq, mc)] = ps
                return ps

            def proj_mm2(qq, mc, casts_on_scalar=False, cast_eng=None, dma_eng=None):
                """Second (ots, K=64) half + cast + output DMA."""
                ps = tail_ps.pop((qq, mc))
                nc.tensor.matmul(
                    ps[:],
                    wprojT[0:64, 768 + mc * 128 : 768 + (mc + 1) * 128],
                    ots[0:64, ts(qq, 512)],
                    start=False, stop=True,
                )
                zb = zpool.tile([128, 512], BF16, tag="zb")
                # casts_on_scalar: alternate scalar/DVE so the tail casts
                # drain in parallel on two engines
                if cast_eng is not None:
                    if cast_eng is nc.scalar:
                        nc.scalar.copy(zb[:], ps[:])
                    else:
                        cast_eng.tensor_copy(zb[:], ps[:])
                elif casts_on_scalar and mc % 2 == 0:
                    nc.scalar.copy(zb[:], ps[:])
                else:
                    nc.vector.tensor_copy(zb[:], ps[:])
                # alternate output queues so the tail's serial DIRECT2D
                # issuance (~0.6us each) halves
                if dma_eng is None:
                    dma_eng = nc.sync if mc % 2 == 0 else nc.gpsimd
                dma_eng.dma_start(
                    out=zT_d[:, (mc * 4 + qq) * 512 : (mc * 4 + qq + 1) * 512],
                    in_=zb[:],
                )

            def proj(qq, casts_on_scalar=False, use_pst=False):
                for mc in range(NDC):
                    proj_mm1(qq, mc, use_pst=use_pst)
                    proj_mm2(qq, mc, casts_on_scalar=casts_on_scalar)

            # proj(qq-1) is spread 2-chains-at-a-time between the qq groups:
            # the PE then has queued work to run while each group's
            # normalization chain (~2us on DVE) resolves.
            for k, (h, qq) in enumerate(groups_order):
                if h == 2 and qq == 3:
                    # last group, inlined: finish proj(2), run the first PV
                    # pair, THEN pre-emit the otp-half of the last
                    # projection (it waits on h1q3's normalize - emitting it
                    # after this group's first PVs keeps the in-order PE
                    # queue fed while that resolves).  pmc/pst tiles only
                    # (NOT pot: its bufs are needed by this group's
                    # accumulator and the PE queue would deadlock on the
                    # recycle).  Only the K=64 halves remain after the final
                    # normalize.
                    for mc in (4, 5):
                        proj_mm1(2, mc)
                        proj_mm2(2, mc, casts_on_scalar=True)
                    ot = pot.tile([128, 512], F32, tag="ot")
                    jobs = pre_jobs.pop((h, qq))
                    emit_pv(h, ot, jobs[0])
                    emit_pv(h, ot, jobs[1])
                    for mc in (0, 1, 3, 4):
                        proj_mm1(3, mc, use_pst=True)  # pmc/pst tiles
                    dt = invpool.tile([64, 1024], F32, tag="dt")
                    norm_span(h, ot, dt, qq * 512, 0, 512, 512)
                    break
                attn_group(h, qq, min(k + 2, len(groups_order) - 1))
                # drain any stragglers for the next group before its PVs
                emit_next_s(3, min(k + 1, len(groups_order) - 1), limit=99)
                if qq > 0 and not (h == 2 and qq == 3):
                    for mc in (2 * h, 2 * h + 1):
                        proj_mm1(qq - 1, mc)
                        proj_mm2(qq - 1, mc, casts_on_scalar=(qq == 3))
            # mm1(3,2) recycles ot(h1q3) - independent of the final
            # normalize, so it runs while that resolves.  Tail casts spread
            # over scalar/gpsimd/vector and DMA issuance over sync/gpsimd so
            # no single engine serializes the drain; the last chain (mc5)
            # gets the least-loaded engines.
            # (gpsimd cannot read PSUM, so casts go scalar/vector only;
            # scalar takes the final chain, vector is free after the norm)
            tail_eng = {
                0: (nc.scalar, nc.sync),
                1: (nc.vector, nc.gpsimd),
                2: (nc.scalar, nc.sync),
                3: (nc.vector, nc.gpsimd),
                4: (nc.scalar, nc.sync),
                5: (nc.scalar, nc.sync),
            }
            proj_mm1(3, 2, use_pst=True)           # pot tile
            for mc in (0, 1, 2, 3, 4):
                proj_mm2(3, mc, cast_eng=tail_eng[mc][0], dma_eng=tail_eng[mc][1])
            proj_mm1(3, 5, use_pst=True)           # pot tile
            proj_mm2(3, 5, cast_eng=tail_eng[5][0], dma_eng=tail_eng[5][1])

    nc.compile()
    return nc


@functools.lru_cache(maxsize=1)
def get_nc():
    return build_nc()


def make_in_maps(x, qkv_w, qkv_b, proj_w):
    """Per-core host-side sharding/layout prep."""
    x = np.asarray(x, dtype=np.float32)
    qkv_w = np.asarray(qkv_w, dtype=np.float32)
    qkv_b = np.asarray(qkv_b, dtype=np.float32)
    proj_w = np.asarray(proj_w, dtype=np.float32)

    # x host layout: [128, nt, half, dc(3), 512] flattened, per batch
    x_pm = []
    for b in range(B):
        xT = np.ascontiguousarray(x[b].reshape(L, D).T)          # (768, 2048)
        arr = xT.reshape(NDC, 128, 4, 512).transpose(1, 2, 0, 3)  # (128, 4, 6, 512)
        x_pm.append(
            np.ascontiguousarray(arr.reshape(128, 12288)).astype(ml_dtypes.bfloat16)
        )

    in_maps = []
    for c in range(8):
        b, g = divmod(c, 4)
        h0, h1, h2 = 3 * g, 3 * g + 1, 3 * g + 2

        def qrows(h):
            return slice(h * HD, (h + 1) * HD)

        def krows(h):
            return slice(D + h * HD, D + (h + 1) * HD)

        def vrows(h):
            return slice(2 * D + h * HD, 2 * D + (h + 1) * HD)

        # qk selection: mc0=[q0|q1] mc1=[k0|k1] mc2=[q2|k2]
        order = [
            qrows(h0), qrows(h1), krows(h0), krows(h1), qrows(h2), krows(h2),
        ]
        wqk = np.concatenate([qkv_w[s] for s in order], axis=0)       # (384, 768)
        # DRAM layout: K block first, then Q, then qk2 (DMA priority order)
        wqkT_host = np.concatenate(
            [
                pmajor(wqk[128:256].T, 128),   # [k0|k1]
                pmajor(wqk[0:128].T, 128),     # [q0|q1]
                pmajor(wqk[256:384].T, 128),   # [q2|k2]
            ],
            axis=1,
        )
        bqk_sel = np.concatenate([qkv_b[s] for s in order], axis=0)   # (384,)
        bcol = np.zeros((128, 4), np.float32)
        for mc in range(3):
            bcol[:, mc] = bqk_sel[mc * 128 : (mc + 1) * 128]
        # natural-V weights: 3 heads x 64 cols (ones slot is on-chip static)
        wv3 = np.concatenate(
            [qkv_w[vrows(h)] for h in (h0, h1, h2)], axis=0
        )  # (192, 768)
        bv3 = np.concatenate([qkv_b[vrows(h)] for h in (h0, h1, h2)], axis=0)
        wpp = np.concatenate(
            [proj_w[:, ts_np(h0)].T, proj_w[:, ts_np(h1)].T], axis=0
        )  # (128, 768)
        wps = np.concatenate(
            [proj_w[:, ts_np(h2)].T, np.zeros((64, D), np.float32)], axis=0
        )  # (128, 768)
        in_maps.append(
            {
                "xT": x_pm[b],
                "wqkT": wqkT_host,
                "wvT3": pmajor(wv3.T, VW),
                "bqk": bcol,
                "bvb": np.broadcast_to(bv3, (128, VW)).copy(),
                "wprojT": np.ascontiguousarray(
                    np.concatenate([wpp, wps], axis=1)
                ).astype(ml_dtypes.bfloat16),
            }
        )
    return in_maps


def ts_np(h):
    return slice(h * HD, (h + 1) * HD)


def pmajor(wT, width):
    """[768, width] contraction-major -> [128, 6*width] partition-major."""
    w3 = np.asarray(wT, np.float32).reshape(NDC, 128, width)
    return np.ascontiguousarray(
        w3.transpose(1, 0, 2).reshape(128, NDC * width)
    ).astype(ml_dtypes.bfloat16)


def unpack_zT(zt):
    """[128, 12288] device layout -> z^T [768, 2048]."""
    return (
        np.asarray(zt)
        .reshape(128, NDC, 4, 512)
        .transpose(1, 0, 2, 3)
        .reshape(D, L)
    )


def assemble_output(results, proj_b):
    proj_b = np.asarray(proj_b, dtype=np.float32)
    out = np.zeros((B, L, D), np.float32)
    for c in range(8):
        b = c // 4
        out[b] += unpack_zT(results[c]["zT"].astype(np.float32)).T
    out += proj_b[None, None, :]
    return out.reshape(B, T, N, D)


def _install_ntff_hook():
    """The container's antenv stub lacks axon_hooks; recreate it from the
    boot helper so trace=True can profile through libaxon_pjrt."""
    import sys
    import types

    try:
        from antenv.axon_hooks import get_axon_ntff_profile_hook  # noqa: F401

        return
    except ImportError:
        pass
    import antenv
    from trn_agent_boot.trn_boot import _ntff_profile_via_ctypes

    state = {"hook": _ntff_profile_via_ctypes("/opt/axon/libaxon_pjrt.so")}
    mod = types.ModuleType("antenv.axon_hooks")
    mod.set_axon_ntff_profile_hook = lambda h: state.__setitem__("hook", h)
    mod.get_axon_ntff_profile_hook = lambda: state["hook"]
    sys.modules["antenv.axon_hooks"] = mod
    antenv.axon_hooks = mod

    import concourse.bass_utils as bu

    orig_upload = bu.upload_artifacts

    def safe_upload(tmpdir):
        try:
            return orig_upload(tmpdir)
        except Exception:
            return tmpdir

    bu.upload_artifacts = safe_upload


def kernel_with_stats(x, qkv_w, qkv_b, proj_w, proj_b, trace=False):
    from concourse.bass_utils import run_bass_kernel_spmd

    if trace:
        _install_ntff_hook()
    nc = get_nc()
    in_maps = make_in_maps(x, qkv_w, qkv_b, proj_w)
    res = run_bass_kernel_spmd(nc, in_maps, list(range(8)), trace=trace)
    return assemble_output(res.results, proj_b), res


def kernel(x, qkv_w, qkv_b, proj_w, proj_b):
    out, _ = kernel_with_stats(x, qkv_w, qkv_b, proj_w, proj_b)
    return out


# revision 54
# speedup vs baseline: 1.0255x; 1.0158x over previous
"""Block-causal (anti-causal: key-block >= query-block) multi-head attention
for Trainium2, run SPMD on 8 NeuronCores.

Problem (hardcoded): B=2, T=8, N=256 (L=2048), D=768, H=12, HD=64.
reference:
    qkv = x @ qkv_w.T + qkv_b ; split into q,k,v heads
    s   = (q @ k.T) / 8 ; mask: query in block ti attends keys in blocks tj >= ti
    p   = softmax(s) ; y = p @ v ; out = y @ proj_w.T + proj_b

Sharding: data-parallel over B (2) x tensor-parallel over heads (4 groups of
3 heads) = 8 cores. Each core computes, for its (batch, head-group):
  - Q^T,K^T = Wqk @ x^T   (bf16 matmuls; bias folded into the PSUM->SBUF
              copy on the DVE: tensor_scalar_add)
  - Vn      = x-chunk.T @ Wv-stream: V in NATURAL layout [keys, 4 slots of
              64] per 128-key chunk (xT chunk stationary, V weights moving).
              Slot 3 is a static all-ones block (memset once): every PV
              matmul's stationary is [v_h | ones] = 128 cols, so PSUM rows
              64:127 of the attention accumulator hold the softmax
              denominator replicated 64x (no partition-broadcast needed).
  - S^T     = K^T.T-chunks vs Q^T   (keys on partitions, queries on free dim)
  - P~      = exp(0.125 * S^T)      (no max-subtraction; logits are tiny)
  - U^T     = [Vn | ones].T @ P~     (rows 0:63 U, rows 64:127 denominator)
  - O^T     = U^T * recip(den) on DVE only (copy den PSUM->SBUF, fast
              reciprocal, broadcast-free multiply)
  - Z^T    += Wproj-slice @ O^T      (partial projection output, bf16)
Host sums the 4 head-group partials per batch and adds proj_b.

DMA: all DRAM tensors are partition-major with per-partition-contiguous
blocks per transfer (cheap DIRECT2D issuance).  Three queues issue in
parallel: sync (x low-dc halves, output), scalar (wqk K/Q + bqk, issued
before its activation-table load), gpsimd (x high-dc halves, wv, bvb,
wqk2, wproj).  K/Q weights and x nt0 land first so the qk chains start
~10us in (vs ~15us with 2-queue serial issuance).

Schedule: K/Q chains for nt0 run first, then the nt0 V-chains and the nt0
qk2 chain (x0-only work that fills the PE while x1..x3 stream); the
remaining qk2/V chains are interleaved with pre-emitted S+exp key-chunk
pairs.  EVERY attention group's S pairs are emitted ~2 groups ahead of
its PV matmuls (uniform lookahead, throttled by the 2-buffer S-tile PSUM
ring).  Key chunks are reordered so the first PV matmul of each group
covers the full 512-query PSUM bank with start=True (pending-zero is
bank-granular).  The masked pair's two 256-query S segments share one
PSUM bank (single accumulation group) so one exp covers both.  The
projection for quarter qq is spread 2-chains-at-a-time between quarter
qq+1's attention groups (the PE then has queued work while each group's
normalization chain resolves on DVE); the otp-half of the last projection
is pre-emitted on the free pmc/pst banks before the final group so only
the K=64 ots-halves + casts + output DMAs (alternating sync/gpsimd
queues) remain after the final normalize.

Empirical notes (HW-verified): a partition-base-shifted custom-DVE
reciprocal output writes garbage (keep recip in/out on one partition
base, split by columns); a K=128 zero-padded ots-half projection is
SLOWER than the K=64 one; pre-emitting a pot-pool proj tile before the
last attention group head-blocks the in-order PE queue.
"""

import functools

import ml_dtypes
import numpy as np

import concourse.bass as bass
import concourse.bacc as bacc_mod
import concourse.mybir as mybir
import concourse.tile as tile
from concourse.bass import ts

F32 = mybir.dt.float32
BF16 = mybir.dt.bfloat16

B, T, N, D = 2, 8, 256, 768
H, HD = 12, 64
L = T * N          # 2048
HPC = 3            # heads per core
NKC = L // 128     # 16 key chunks of 128
NDC = D // 128     # 6 contraction chunks
SCALE = 1.0 / 8.0
VW = 192           # natural-V row width: [v_h0 | v_h1 | v_h2] (ones separate)


DEBUG = False


def group_pairs(qq):
    """Key-chunk pairs for one (head, qq) group; masked pair last."""
    kcs = list(range(4 * qq + 2, 16)) + [4 * qq, 4 * qq + 1]
    return [(kcs[i], kcs[i + 1]) for i in range(0, len(kcs), 2)]


def build_nc():
    nc = bacc_mod.Bacc()

    # x: [128, nt(4), half(2), dc(3), 512] flattened -> per-transfer 3KB
    # contiguous per partition
    xT_d = nc.declare_dram_parameter("xT", [128, 12288], BF16, isOutput=False)
    # wqk: per block (K|Q|qk2) [dc(6), 128] contiguous per partition
    wqkT_d = nc.declare_dram_parameter("wqkT", [128, 2304], BF16, isOutput=False)
    wvT3_d = nc.declare_dram_parameter("wvT3", [128, NDC * VW], BF16, isOutput=False)
    bqk_d = nc.declare_dram_parameter("bqk", [128, 4], F32, isOutput=False)
    bvb_d = nc.declare_dram_parameter("bvb", [128, VW], F32, isOutput=False)
    wprojT_d = nc.declare_dram_parameter("wprojT", [128, 1536], BF16, isOutput=False)
    # z: [128, mc(6), qq(4), 512] flattened
    zT_d = nc.declare_dram_parameter("zT", [128, 12288], BF16, isOutput=True)
    if DEBUG:
        dbg_vn = nc.declare_dram_parameter("dbg_vn", [128, 384], BF16, isOutput=True)
        dbg_qt = nc.declare_dram_parameter("dbg_qt", [128, 512], BF16, isOutput=True)
        dbg_kt = nc.declare_dram_parameter("dbg_kt", [128, 512], BF16, isOutput=True)
        dbg_dt = nc.declare_dram_parameter("dbg_dt", [128, 512], F32, isOutput=True)
        dbg_ot = nc.declare_dram_parameter("dbg_ot", [128, 512], BF16, isOutput=True)

    with tile.TileContext(nc) as tc:
        with (
            tc.tile_pool(name="persist", bufs=1) as pp,
            tc.tile_pool(name="ptile", bufs=26) as ppool,
            tc.tile_pool(name="zbuf", bufs=6) as zpool,
            tc.tile_pool(name="invb", bufs=2) as invpool,
            tc.tile_pool(name="psum_st", bufs=2, space="PSUM") as pst,
            tc.tile_pool(name="psum_ot", bufs=2, space="PSUM") as pot,
            tc.tile_pool(name="psum_mc", bufs=2, space="PSUM") as pmc,
        ):
            # ---- persistent SBUF tensors ----
            wqkT = pp.tile([128, 3, NDC, 128], BF16, tag="wqkT")
            wvT3 = pp.tile([128, NDC, VW], BF16, tag="wvT3")
            bqk = pp.tile([128, 4], F32, tag="bqk")
            bvb = pp.tile([128, VW], F32, tag="bvb")
            wprojT = pp.tile([128, 1536], BF16, tag="wprojT")
            qt = pp.tile([128, L], BF16, tag="qt")      # [q_h0 | q_h1]
            kt = pp.tile([128, L], BF16, tag="kt")      # [k_h0 | k_h1]
            qk2 = pp.tile([128, L], BF16, tag="qk2")    # [q_h2 | k_h2]
            kt2 = pp.tile([64, L], BF16, tag="kt2")     # k_h2 re-based to part 0
            # per key chunk: [v_h0 |ones| v_h1 |ones| v_h2 |ones] so each
            # head's PV stationary [v_h | ones] is one contiguous 128-col
            # slice (the compiler requires single-free-dim weight APs)
            vn = pp.tile([128, NKC, 384], BF16, tag="vn")
            otp = pp.tile([128, L], BF16, tag="otp")    # [o_h0 | o_h1]
            ots = pp.tile([64, L], BF16, tag="ots")     # [o_h2]

            qt_src = [qt[0:64, :], qt[64:128, :], qk2[0:64, :]]
            kt_src = [kt[0:64, :], kt[64:128, :], kt2[0:64, :]]
            ot_dst = [otp[0:64, :], otp[64:128, :], ots[0:64, :]]
            scratch = pp.tile([128, 512], BF16, tag="scratch")

            def pe_warm(n):
                """K=128 dummy matmuls: keep the PE's utilization-driven
                clock ramped across a known stall.  Only safe when no input
                DMA is streaming (the K=128 SBUF reads throttle DMA)."""
                for _ in range(n):
                    ds = pmc.tile([128, 512], F32, tag="qs")
                    nc.tensor.matmul(
                        ds[:], scratch[:, 0:128], scratch[:],
                        start=True, stop=True,
                    )

            def vap(h, kc):
                """PV stationary: [v_h | ones], one contiguous 128-col slice."""
                return vn[:, kc, 128 * h : 128 * h + 128]

            def emit_s_pair(h, qq, pi, a, b):
                """S matmuls + exp for one key-chunk pair; returns a PV job."""
                q_lo = qq * 512
                masked = a == 4 * qq
                st2 = pst.tile([128, 1024], F32, tag="st")
                pt = ppool.tile([128, 1024], BF16, tag="pt")
                if masked:
                    # both 256-query segments adjacent in one PSUM bank ->
                    # a single exp.  One accumulation group (disjoint
                    # regions): pending-zero is bank-granular, so a second
                    # start=True would clobber the first segment.
                    nc.tensor.matmul(
                        st2[:, 0:256],
                        kt_src[h][:, ts(a, 128)],
                        qt_src[h][:, q_lo : q_lo + 256],
                        start=True, stop=False, skip_group_check=True,
                    )
                    nc.tensor.matmul(
                        st2[:, 256:512],
                        kt_src[h][:, ts(b, 128)],
                        qt_src[h][:, q_lo : q_lo + 256],
                        start=False, stop=True, skip_group_check=True,
                    )
                    nc.scalar.activation(
                        pt[:, 0:512],
                        st2[:, 0:512],
                        mybir.ActivationFunctionType.Exp,
                        scale=SCALE,
                    )
                else:
                    nc.tensor.matmul(
                        st2[:, 0:512],
                        kt_src[h][:, ts(a, 128)],
                        qt_src[h][:, q_lo : q_lo + 512],
                        start=True, stop=True,
                    )
                    nc.tensor.matmul(
                        st2[:, 512:1024],
                        kt_src[h][:, ts(b, 128)],
                        qt_src[h][:, q_lo : q_lo + 512],
                        start=True, stop=True,
                    )
                    nc.scalar.activation(
                        pt[:, 0:1024],
                        st2[:, 0:1024],
                        mybir.ActivationFunctionType.Exp,
                        scale=SCALE,
                    )
                return (pi, a, b, pt, masked)

            def emit_pv(h, ot, job):
                pi, a, b, pt, masked = job
                if not masked:
                    nc.tensor.matmul(
                        ot[:, 0:512], vap(h, a), pt[:, 0:512],
                        start=(pi == 0), stop=False, skip_group_check=True,
                    )
                    nc.tensor.matmul(
                        ot[:, 0:512], vap(h, b), pt[:, 512:1024],
                        start=False, stop=False, skip_group_check=True,
                    )
                else:
                    nc.tensor.matmul(
                        ot[:, 0:256], vap(h, a), pt[:, 0:256],
                        start=False, stop=False, skip_group_check=True,
                    )
                    nc.tensor.matmul(
                        ot[:, 0:256], vap(h, b), pt[:, 256:512],
                        start=False, stop=True, skip_group_check=True,
                    )

            # ---- input DMAs ----
            # Three issuing queues in parallel (sync + scalar are HWDGE,
            # gpsimd SWDGE).  Every transfer is per-partition contiguous in
            # DRAM.  Priority: x nt0 + K block + Q block land first.
            with tc.tile_pool(name="xT", bufs=1) as xp:
                xT = xp.tile([128, NDC, L], BF16, tag="xT")

                def x_dma(eng, nt, dc0, dc1):
                    eng.dma_start(
                        out=xT[:, dc0:dc1, ts(nt, 512)],
                        in_=xT_d[:, (nt * 6 + dc0) * 512 : (nt * 6 + dc1) * 512]
                        .rearrange("p (dc w) -> p dc w", w=512),
                    )

                def w_dma(blk, mc):
                    nc.gpsimd.dma_start(
                        out=wqkT[:, mc],
                        in_=wqkT_d[:, 768 * blk : 768 * (blk + 1)]
                        .rearrange("p (dc w) -> p dc w", w=128),
                    )

                # K then Q blocks + bias on the scalar HWDGE queue, issued
                # before the activation-table load (a 3-way x0 split
                # regresses: measured twice)
                x_dma(nc.sync, 0, 0, 3)
                x_dma(nc.gpsimd, 0, 3, 6)
                nc.scalar.dma_start(
                    out=wqkT[:, 1],
                    in_=wqkT_d[:, 0:768].rearrange("p (dc w) -> p dc w", w=128),
                )
                nc.scalar.dma_start(
                    out=wqkT[:, 0],
                    in_=wqkT_d[:, 768:1536].rearrange("p (dc w) -> p dc w", w=128),
                )
                nc.scalar.dma_start(out=bqk[:], in_=bqk_d[:, :])
                # wv early: the nt0 V-chains fill the PE while x1..x3 stream
                nc.gpsimd.dma_start(
                    out=wvT3[:],
                    in_=wvT3_d[:, :].rearrange("p (dc w) -> p dc w", w=VW),
                )
                nc.gpsimd.dma_start(out=bvb[:], in_=bvb_d[:, :])
                w_dma(2, 2)
                x_dma(nc.sync, 1, 0, 3)
                x_dma(nc.gpsimd, 1, 3, 6)
                x_dma(nc.sync, 2, 0, 3)
                x_dma(nc.gpsimd, 2, 3, 6)
                x_dma(nc.sync, 3, 0, 3)
                x_dma(nc.gpsimd, 3, 3, 6)
                nc.gpsimd.dma_start(out=wprojT[:], in_=wprojT_d[:, :])
                # static tiles (after DMA issuance so they don't delay it)
                nc.vector.memset(scratch[:], 0.0)
                for off in (64, 192, 320):
                    nc.gpsimd.memset(vn[:, :, off : off + 64], 1.0)
                # Pre-warm the exp table: the scalar engine runs exp-only
                # from here on (all bias copies live on DVE).
                warm = zpool.tile([128, 32], F32, tag="warm")
                nc.vector.memset(warm[:], 0.0)
                nc.scalar.activation(
                    warm[:], warm[:], mybir.ActivationFunctionType.Exp
                )

                # ---- phase 1: qk chains + natural-V chains, interleaved with
                # pre-emitted S+exp pairs. ----
                groups_order = [(h, qq) for qq in range(4) for h in range(HPC)]
                pre_jobs = {hq: [] for hq in groups_order}
                s_slots = [
                    (gi, hq, pi, a, b)
                    for gi, hq in enumerate(groups_order)
                    for pi, (a, b) in enumerate(group_pairs(hq[1]))
                ]
                s_done = 0
                kt2_emitted = False

                def emit_next_s(ready_nt, max_gi, limit=1):
                    """Emit queued S pairs whose inputs have landed (kt is
                    written nt-progressively; h2 groups need the kt2 rebase
                    DMA emitted first), up to group index max_gi."""
                    nonlocal s_done
                    while s_done < len(s_slots) and limit > 0:
                        gi, hq, pi, a, b = s_slots[s_done]
                        if gi > max_gi:
                            return
                        if max(a, b) >= 4 * (ready_nt + 1):
                            return
                        if hq[0] == 2 and not kt2_emitted:
                            return
                        pre_jobs[hq].append(emit_s_pair(hq[0], hq[1], pi, a, b))
                        s_done += 1
                        limit -= 1

                def qk_chain(mc, dst, nt):
                    ps = pmc.tile([128, 512], F32, tag="qs")
                    for dc in range(NDC):
                        nc.tensor.matmul(
                            ps[:],
                            wqkT[:, mc, dc, :],
                            xT[:, dc, ts(nt, 512)],
                            start=(dc == 0),
                            stop=(dc == NDC - 1),
                        )
                    nc.vector.tensor_scalar_add(
                        dst[:, ts(nt, 512)], ps[:], bqk[:, mc : mc + 1]
                    )

                def v_chain(kc):
                    vp = pot.tile([128, 256], F32, tag="ot")
                    for dc in range(NDC):
                        nc.tensor.matmul(
                            vp[:, 0:VW],
                            xT[:, dc, ts(kc, 128)],
                            wvT3[:, dc, :],
                            start=(dc == 0),
                            stop=(dc == NDC - 1),
                        )
                    nc.vector.tensor_tensor(
                        out=vn[:, kc, :].rearrange(
                            "p (h s) -> p h s", s=128
                        )[:, :, 0:64],
                        in0=vp[:, 0:VW].rearrange("p (h s) -> p h s", s=64),
                        in1=bvb[:].rearrange("p (h s) -> p h s", s=64),
                        op=mybir.AluOpType.add,
                    )

                def qk2_chain(nt):
                    # qk2 chain (bias on DVE like the rest)
                    ps = pmc.tile([128, 512], F32, tag="qs")
                    for dc in range(NDC):
                        nc.tensor.matmul(
                            ps[:],
                            wqkT[:, 2, dc, :],
                            xT[:, dc, ts(nt, 512)],
                            start=(dc == 0),
                            stop=(dc == NDC - 1),
                        )
                    nc.vector.tensor_scalar_add(
                        qk2[:, ts(nt, 512)], ps[:], bqk[:, 2:3]
                    )

                # small warm-up matmuls (only need the scratch memset): start
                # the PE's HAM activity window ~1us before x0 lands so the
                # first chains reach the 2.4GHz clock sooner.  N=256 keeps
                # the SBUF read traffic low (large dummies throttle the
                # input DMA stream - measured in an earlier session).
                for _ in range(16):
                    ds = pmc.tile([128, 512], F32, tag="qs")
                    nc.tensor.matmul(
                        ds[:, 0:256], scratch[:, 0:128], scratch[:, 0:256],
                        start=True, stop=True,
                    )
                qk_chain(1, kt, 0)
                qk_chain(0, qt, 0)
                # nt0 V-chains + nt0 qk2 chain need only x0 (+wv/wqk2):
                # they fill the PE while x1..x3 stream in
                for kc in range(4):
                    v_chain(kc)
                qk2_chain(0)
                for nt in range(1, 4):
                    qk_chain(1, kt, nt)       # keys: S pairs consume these
                    emit_next_s(nt - 1, 2, limit=2)
                    qk_chain(0, qt, nt)
                    emit_next_s(nt - 1, 2, limit=2)
                for nt in range(1, 4):
                    qk2_chain(nt)
                    if nt == 3:
                        # k_h2 re-base: partitions 64:128 -> 0:64
                        nc.gpsimd.dma_start(out=kt2[0:64, :], in_=qk2[64:128, :])
                        kt2_emitted = True
                    for kc in range(4 * nt, 4 * nt + 4):
                        v_chain(kc)
                        emit_next_s(3, 2)
                    emit_next_s(3, 2)
                emit_next_s(3, 2, limit=99)  # drain groups 0..2 leftovers
                if DEBUG:
                    nc.sync.dma_start(out=dbg_vn[:, :], in_=vn[:, 0, :])
                    nc.sync.dma_start(out=dbg_qt[:, :], in_=qt[:, 0:512])
                    nc.sync.dma_start(out=dbg_kt[:, :], in_=kt[:, 0:512])

            # ---- attention + interleaved projection ----
            def norm_span(h, ot, dt, q_lo, c0, c1, r0):
                """Normalize ot cols [c0:c1] -> ot_dst cols [q_lo+c0 ...].
                recip in/out must share the partition base on HW (a
                partition-shifted custom-DVE output writes garbage), so the
                inverse goes to a column-offset scratch region [r0...]."""
                nc.vector.tensor_copy(dt[0:64, c0:c1], ot[64:128, c0:c1])
                nc.vector.reciprocal_approx_fast(
                    dt[0:64, r0 : r0 + (c1 - c0)], dt[0:64, c0:c1]
                )
                nc.vector.tensor_tensor(
                    out=ot_dst[h][:, q_lo + c0 : q_lo + c1],
                    in0=ot[0:64, c0:c1],
                    in1=dt[0:64, r0 : r0 + (c1 - c0)],
                    op=mybir.AluOpType.mult,
                )

            def attn_group(h, qq, lookahead_gi):
                ot = pot.tile([128, 512], F32, tag="ot")
                jobs = pre_jobs.pop((h, qq))
                assert len(jobs) == len(group_pairs(qq))
                q_lo = qq * 512
                dt = invpool.tile([64, 1024], F32, tag="dt")
                for job in jobs:
                    emit_pv(h, ot, job)
                    # one lookahead S pair (group +2) per PV slot
                    emit_next_s(3, lookahead_gi, limit=1)
                # normalize: PSUM rows 64:127 hold den replicated; copy to
                # SBUF (custom-DVE recip reading PSUM returns garbage on HW),
                # reciprocal, broadcast-free multiply.  All on DVE.
                norm_span(h, ot, dt, q_lo, 0, 512, 512)
                if DEBUG and h == 0 and qq == 0:
                    nc.sync.dma_start(out=dbg_dt[:, :], in_=dt[:])
                    nc.sync.dma_start(out=dbg_ot[:, :], in_=otp[:, 0:512])

            tail_ps = {}

            def proj_mm1(qq, mc, use_pst=False):
                """First (otp, K=128) half of the mc-th projection chain."""
                if use_pst:
                    # tail: S tiles and attention accumulators are done;
                    # rotate over all three pools (6 banks) so the casts
                    # never gate the next chain
                    if mc % 3 == 0:
                        ps = pmc.tile([128, 512], F32, tag="qs")
                    elif mc % 3 == 1:
                        pst_tile = pst.tile([128, 1024], F32, tag="st", name="pst_tile")
                        ps = pst_tile[:, 0:512]
                    else:
                        ps = pot.tile([128, 512], F32, tag="ot")
                else:
                    # both interleaved proj tiles live on pmc (2 allocations
                    # per group boundary, bufs=2 -> recycle exactly one
                    # boundary back, consumers already drained).  pot then
                    # holds ONLY the attention accumulators in phase 2, so
                    # ot(k+1) recycles ot(k-1) and never waits on the ~2us
                    # normalize of the group right before it.
                    ps = pmc.tile([128, 512], F32, tag="qs")
                nc.tensor.matmul(
                    ps[:],
                    wprojT[:, ts(mc, 128)],
                    otp[:, ts(qq, 512)],
                    start=True, stop=False,
                )
                tail_ps[(qq, mc)] = ps
                return ps

            def proj_mm2(qq, mc, casts_on_scalar=False, cast_eng=None, dma_eng=None):
                """Second (ots, K=64) half + cast + output DMA."""
                ps = tail_ps.pop((qq, mc))
                nc.tensor.matmul(
                    ps[:],
                    wprojT[0:64, 768 + mc * 128 : 768 + (mc + 1) * 128],
                    ots[0:64, ts(qq, 512)],
                    start=False, stop=True,
                )
                zb = zpool.tile([128, 512], BF16, tag="zb")
                # casts_on_scalar: alternate scalar/DVE so the tail casts
                # drain in parallel on two engines
                if cast_eng is not None:
                    if cast_eng is nc.scalar:
                        nc.scalar.copy(zb[:], ps[:])
                    else:
                        cast_eng.tensor_copy(zb[:], ps[:])
                elif casts_on_scalar and mc % 2 == 0:
                    nc.scalar.copy(zb[:], ps[:])
                else:
                    nc.vector.tensor_copy(zb[:], ps[:])
                # alternate output queues so the tail's serial DIRECT2D
                # issuance (~0.6us each) halves
                if dma_eng is None:
                    dma_eng = nc.sync if mc % 2 == 0 else nc.gpsimd
                dma_eng.dma_start(
                    out=zT_d[:, (mc * 4 + qq) * 512 : (mc * 4 + qq + 1) * 512],
                    in_=zb[:],
                )

            def proj(qq, casts_on_scalar=False, use_pst=False):
                for mc in range(NDC):
                    proj_mm1(qq, mc, use_pst=use_pst)
                    proj_mm2(qq, mc, casts_on_scalar=casts_on_scalar)

            # proj(qq-1) is spread 2-chains-at-a-time between the qq groups:
            # the PE then has queued work to run while each group's
            # normalization chain (~2us on DVE) resolves.
            for k, (h, qq) in enumerate(groups_order):
                if h == 2 and qq == 3:
                    # last group, inlined: finish proj(2), run the first PV
                    # pair, THEN pre-emit the otp-half of the last
                    # projection (it waits on h1q3's normalize - emitting it
                    # after this group's first PVs keeps the in-order PE
                    # queue fed while that resolves).  pmc/pst tiles only
                    # (NOT pot: its bufs are needed by this group's
                    # accumulator and the PE queue would deadlock on the
                    # recycle).  Only the K=64 halves remain after the final
                    # normalize.
                    for mc in (4, 5):
                        proj_mm1(2, mc)
                        proj_mm2(2, mc, casts_on_scalar=True)
                    ot = pot.tile([128, 512], F32, tag="ot")
                    jobs = pre_jobs.pop((h, qq))
                    emit_pv(h, ot, jobs[0])
                    emit_pv(h, ot, jobs[1])
                    for mc in (0, 1, 3, 4):
                        proj_mm1(3, mc, use_pst=True)  # pmc/pst tiles
                    dt = invpool.tile([64, 1024], F32, tag="dt")
                    norm_span(h, ot, dt, qq * 512, 0, 512, 512)
                    break
                attn_group(h, qq, min(k + 2, len(groups_order) - 1))
                # drain any stragglers for the next group before its PVs
                emit_next_s(3, min(k + 1, len(groups_order) - 1), limit=99)
                if qq > 0 and not (h == 2 and qq == 3):
                    for mc in (2 * h, 2 * h + 1):
                        proj_mm1(qq - 1, mc)
                        proj_mm2(qq - 1, mc, casts_on_scalar=(qq == 3))
            # mm1(3,2) recycles ot(h1q3) - independent of the final
            # normalize, so it runs while that resolves.  Tail casts spread
            # over scalar/gpsimd/vector and DMA issuance over sync/gpsimd so
            # no single engine serializes the drain; the last chain (mc5)
            # gets the least-loaded engines.
            # (gpsimd cannot read PSUM, so casts go scalar/vector only;
            # scalar takes the final chain, vector is free after the norm)
            tail_eng = {
                0: (nc.scalar, nc.sync),
                1: (nc.vector, nc.gpsimd),
                2: (nc.scalar, nc.sync),
                3: (nc.vector, nc.gpsimd),
                4: (nc.scalar, nc.sync),
                5: (nc.scalar, nc.sync),
            }
            proj_mm1(3, 2, use_pst=True)           # pot tile
            for mc in (0, 1, 2, 3, 4):
                proj_mm2(3, mc, cast_eng=tail_eng[mc][0], dma_eng=tail_eng[mc][1])
            proj_mm1(3, 5, use_pst=True)           # pot tile
            proj_mm2(3, 5, cast_eng=tail_eng[5][0], dma_eng=tail_eng[5][1])

    nc.compile()
    return nc


@functools.lru_cache(maxsize=1)
def get_nc():
    return build_nc()


def make_in_maps(x, qkv_w, qkv_b, proj_w):
    """Per-core host-side sharding/layout prep."""
    x = np.asarray(x, dtype=np.float32)
    qkv_w = np.asarray(qkv_w, dtype=np.float32)
    qkv_b = np.asarray(qkv_b, dtype=np.float32)
    proj_w = np.asarray(proj_w, dtype=np.float32)

    # x host layout: [128, nt, half, dc(3), 512] flattened, per batch
    x_pm = []
    for b in range(B):
        xT = np.ascontiguousarray(x[b].reshape(L, D).T)          # (768, 2048)
        arr = xT.reshape(NDC, 128, 4, 512).transpose(1, 2, 0, 3)  # (128, 4, 6, 512)
        x_pm.append(
            np.ascontiguousarray(arr.reshape(128, 12288)).astype(ml_dtypes.bfloat16)
        )

    in_maps = []
    for c in range(8):
        b, g = divmod(c, 4)
        h0, h1, h2 = 3 * g, 3 * g + 1, 3 * g + 2

        def qrows(h):
            return slice(h * HD, (h + 1) * HD)

        def krows(h):
            return slice(D + h * HD, D + (h + 1) * HD)

        def vrows(h):
            return slice(2 * D + h * HD, 2 * D + (h + 1) * HD)

        # qk selection: mc0=[q0|q1] mc1=[k0|k1] mc2=[q2|k2]
        order = [
            qrows(h0), qrows(h1), krows(h0), krows(h1), qrows(h2), krows(h2),
        ]
        wqk = np.concatenate([qkv_w[s] for s in order], axis=0)       # (384, 768)
        # DRAM layout: K block first, then Q, then qk2 (DMA priority order)
        wqkT_host = np.concatenate(
            [
                pmajor(wqk[128:256].T, 128),   # [k0|k1]
                pmajor(wqk[0:128].T, 128),     # [q0|q1]
                pmajor(wqk[256:384].T, 128),   # [q2|k2]
            ],
            axis=1,
        )
        bqk_sel = np.concatenate([qkv_b[s] for s in order], axis=0)   # (384,)
        bcol = np.zeros((128, 4), np.float32)
        for mc in range(3):
            bcol[:, mc] = bqk_sel[mc * 128 : (mc + 1) * 128]
        # natural-V weights: 3 heads x 64 cols (ones slot is on-chip static)
        wv3 = np.concatenate(
            [qkv_w[vrows(h)] for h in (h0, h1, h2)], axis=0
        )  # (192, 768)
        bv3 = np.concatenate([qkv_b[vrows(h)] for h in (h0, h1, h2)], axis=0)
        wpp = np.concatenate(
            [proj_w[:, ts_np(h0)].T, proj_w[:, ts_np(h1)].T], axis=0
        )  # (128, 768)
        wps = np.concatenate(
            [proj_w[:, ts_np(h2)].T, np.zeros((64, D), np.float32)], axis=0
        )  # (128, 768)
        in_maps.append(
            {
                "xT": x_pm[b],
                "wqkT": wqkT_host,
                "wvT3": pmajor(wv3.T, VW),
                "bqk": bcol,
                "bvb": np.broadcast_to(bv3, (128, VW)).copy(),
                "wprojT": np.ascontiguousarray(
                    np.concatenate([wpp, wps], axis=1)
                ).astype(ml_dtypes.bfloat16),
            }
        )
    return in_maps


def ts_np(h):
    return slice(h * HD, (h + 1) * HD)


def pmajor(wT, width):
    """[768, width] contraction-major -> [128, 6*width] partition-major."""
    w3 = np.asarray(wT, np.float32).reshape(NDC, 128, width)
    return np.ascontiguousarray(
        w3.transpose(1, 0, 2).reshape(128, NDC * width)
    ).astype(ml_dtypes.bfloat16)


def unpack_zT(zt):
    """[128, 12288] device layout -> z^T [768, 2048]."""
    return (
        np.asarray(zt)
        .reshape(128, NDC, 4, 512)
        .transpose(1, 0, 2, 3)
        .reshape(D, L)
    )


def assemble_output(results, proj_b):
    proj_b = np.asarray(proj_b, dtype=np.float32)
    out = np.zeros((B, L, D), np.float32)
    for c in range(8):
        b = c // 4
        out[b] += unpack_zT(results[c]["zT"].astype(np.float32)).T
    out += proj_b[None, None, :]
    return out.reshape(B, T, N, D)


def _install_ntff_hook():
    """The container's antenv stub lacks axon_hooks; recreate it from the
    boot helper so trace=True can profile through libaxon_pjrt."""
    import sys
    import types

    try:
        from antenv.axon_hooks import get_axon_ntff_profile_hook  # noqa: F401

        return
    except ImportError:
        pass
    import antenv
    from trn_agent_boot.trn_boot import _ntff_profile_via_ctypes

    state = {"hook": _ntff_profile_via_ctypes("/opt/axon/libaxon_pjrt.so")}
    mod = types.ModuleType("antenv.axon_hooks")
    mod.set_axon_ntff_profile_hook = lambda h: state.__setitem__("hook", h)
    mod.get_axon_ntff_profile_hook = lambda: state["hook"]
    sys.modules["antenv.axon_hooks"] = mod
    antenv.axon_hooks = mod

    import concourse.bass_utils as bu

    orig_upload = bu.upload_artifacts

    def safe_upload(tmpdir):
        try:
            return orig_upload(tmpdir)
        except Exception:
            return tmpdir

    bu.upload_artifacts = safe_upload


def kernel_with_stats(x, qkv_w, qkv_b, proj_w, proj_b, trace=False):
    from concourse.bass_utils import run_bass_kernel_spmd

    if trace:
        _install_ntff_hook()
    nc = get_nc()
    in_maps = make_in_maps(x, qkv_w, qkv_b, proj_w)
    res = run_bass_kernel_spmd(nc, in_maps, list(range(8)), trace=trace)
    return assemble_output(res.results, proj_b), res


def kernel(x, qkv_w, qkv_b, proj_w, proj_b):
    out, _ = kernel_with_stats(x, qkv_w, qkv_b, proj_w, proj_b)
    return out
